# revision 37
# baseline (speedup 1.0000x reference)
"""Attention2d SPMD kernel for 8 TRN2 NeuronCores.

Problem (hardcoded): x [4, 768, 32, 32], w_qkv [768, 2304], b_qkv [2304],
w_proj [768, 768], b_proj [768]; 32 heads, head_dim 24.

Sharding: 8 cores = 4 batches x 2 query-halves (512 queries each).
Each core computes k/v for all 1024 positions of its batch (2x duplicated
across the pair of cores sharing a batch) and q/attention/proj for its own
512 query positions. Outputs are disjoint slices -> host gather is pure
concatenation (no collectives). Per-core x is ROTATED on the host so each
core's queries are always columns 0:512 (softmax is permutation-invariant
over keys), which makes the SPMD program identical across cores.

Per-core dataflow (per head-group g of 4 heads):
  k_g = w_k^T x  [128ch_pad, 1024]  (fp16)     q_g = w_q^T x  [128, 512]
  vT  = x^T w_v  [1024pos, 32 heads x (24ch | ones-col | 7 pad)]  (fp16)
  per head h, kt in 8 key-tiles: sT = k_h^T q_h [128k, 512q] -> Exp ->
    oT[128q-tile, 25] += et[:, qt]^T vT_h    (TRANSPOSED attn@v: queries on
    PSUM partitions, head_dim on the free axis -> 25-cycle matmuls; the
    vT ones-column lands the softmax denominator in oT column 24)
  divide: oT[:, 0:24] * (1/denom col) via one broadcast tensor_tensor per
    head (denominator is a per-partition column now - no DRAM bounce)
  tail: PE-transpose oT -> o [c, q] (identity matmul), then
    out^T[q, 768] = o^T W_p + b_p'   with b_p' = b_proj + W_p^T b_v folded
    on the host (exact: attention weights sum to 1). Host transposes out^T.

Precision: fp16 operands everywhere on the PE (1 cyc/row), fp32 PSUM,
denominator division exact fp32.
"""

import os
import numpy as np

import concourse.bacc as bacc
import concourse.bass as bass
import concourse.mybir as mybir
import concourse.tile as tile
from concourse import bass_utils
from concourse.alu_op_type import AluOpType

C = 768
HW = 1024
QP = 512          # queries per core
NH = 32           # heads
HD = 24           # head dim
NG = 8            # head groups (4 heads each, 32-padded rows)
CT = C // 128     # 6 contraction tiles
PT = HW // 128    # 8 position tiles
NQT = QP // 128   # 4 query tiles
SCALE = HD ** -0.5
BF16 = mybir.dt.bfloat16
FP16 = mybir.dt.float16
F32 = mybir.dt.float32


def emit_kernel(tc, outs, ins):
    from contextlib import ExitStack
    nc = tc.nc
    ctx = ExitStack()
    Exp = mybir.ActivationFunctionType.Exp

    big = ctx.enter_context(tc.tile_pool(name="big", bufs=1))
    kqp = ctx.enter_context(tc.tile_pool(name="kqp", bufs=2))
    wgp = ctx.enter_context(tc.tile_pool(name="wgp", bufs=3))
    expp = ctx.enter_context(tc.tile_pool(name="expp", bufs=8))
    smal = ctx.enter_context(tc.tile_pool(name="smal", bufs=2))
    outp = ctx.enter_context(tc.tile_pool(name="outp", bufs=4))
    # PSUM budget (8 banks): sps 2x[128,2,512]=4, gen 2x[128,512]=2,
    # oT 2x[128,4,32]=2.  Tail transpose/proj tiles reuse the sps slots.
    ps_sps = ctx.enter_context(tc.tile_pool(name="ps_sps", bufs=2, space="PSUM"))
    ps_gen = ctx.enter_context(tc.tile_pool(name="ps_gen", bufs=2, space="PSUM"))
    ps_o = ctx.enter_context(tc.tile_pool(name="ps_o", bufs=2, space="PSUM"))

    # ---------------- persistent SBUF tensors ----------------
    x_sb = big.tile([128, CT, HW], FP16)
    wv_sb = big.tile([128, CT, C], FP16)
    wp_sb = big.tile([128, CT, C], FP16)           # w_proj [c,f], c-chunked
    vt_sb = big.tile([128, PT, NH, 32], FP16)      # 2 MB; col HD is ones
    o_sbT = big.tile([128, NQT, NH, HD], FP16)     # divided o^T
    o_c = big.tile([128, CT, NQT, 128], FP16)      # transposed o (c on part)
    bk_sb = big.tile([128, NG], F32)
    bq_sb = big.tile([128, NG], F32)
    bp_bc = big.tile([128, C], F32)                # b_proj' bcast to all part
    ident = big.tile([128, 128], FP16)

    # DMA queues: SP carries ident + x (2 column-half DMAs: q/kA only need
    # cols 0:512, so the PE can start ~2.4us earlier) + the small tensors;
    # Pool carries the weight streams.  One DMA per tensor: each dma_start
    # pays ~1us of SWDGE fixed cost, so per-chunk DMAs serialize the start.
    xv = ins["x"].rearrange("(t p) n -> p t n", p=128)
    wvv = ins["wv"].rearrange("(t p) m -> p t m", p=128)
    nc.scalar.dma_start(out=ident, in_=ins["ident"])
    nc.scalar.dma_start(out=bk_sb, in_=ins["bk"])
    nc.scalar.dma_start(out=bq_sb, in_=ins["bq"])
    nc.scalar.dma_start(out=bp_bc, in_=ins["bp1"].unsqueeze(0).to_broadcast((128, C)))
    nc.sync.dma_start(out=x_sb[:, 0:3, 0:512], in_=xv[:, 0:3, 0:512])
    nc.sync.dma_start(out=x_sb[:, 3:6, 0:512], in_=xv[:, 3:6, 0:512])
    nc.sync.dma_start(out=x_sb[:, :, 512:1024], in_=xv[:, :, 512:1024])
    warm_sb = big.tile([1, 2], F32)
    nc.vector.memset(warm_sb, 0.0)
    nc.scalar.activation(warm_sb[:, 1:2], warm_sb[:, 0:1], Exp, scale=1.0)
    # only vt column 24 (the denominator ones-column) is ever read beyond 0:24
    nc.vector.memset(vt_sb[:, :, :, 24:25], 1.0)
    # keep the PE continuously busy from ~t=2.5us so its p-state ramp
    # completes before the first real matmul
    warm_ps = ps_o.tile([128, 128], F32, tag="ops", name="warm_ps")
    for _ in range(30):
        nc.tensor.matmul(warm_ps, lhsT=ident, rhs=ident,
                         start=True, stop=True, skip_group_check=True)

    def emit_vt_tile(t, pt):
        # vT for heads 16t..16t+16 (dense, N=384) at position tile pt
        vps = ps_gen.tile([128, 512], F32, tag="gen", name="vps")
        for ct in range(CT):
            nc.tensor.matmul(
                vps[:, 0:384],
                lhsT=x_sb[:, ct, pt * 128:(pt + 1) * 128],
                rhs=wv_sb[:, ct, 384 * t:384 * (t + 1)],
                start=(ct == 0), stop=(ct == CT - 1),
            )
        nc.vector.tensor_copy(
            out=vt_sb[:, pt, 16 * t:16 * (t + 1), 0:HD],
            in_=vps[:, 0:384].rearrange("p (h d) -> p h d", d=HD),
        )

    # vT tiles pending emission: one per scores-slot during g0/g1 so the
    # PE never bursts 2+ vt tiles between exps (which would starve the ACT)
    pending_vt = [(0, pt) for pt in range(PT)] + [(1, pt) for pt in range(PT)]

    o_flat = o_sbT.rearrange("p a h d -> p a (h d)")
    partialb = big.tile([128, NQT, 2, 384], F32)   # proj(ct0..4) + bias

    def emit_tp(qt, cts):
        # PE-transpose o^T chunks -> o_c (c on partitions)
        nct = len(cts)
        tp = ps_gen.tile([128, nct, 128], FP16, tag="gen", name="tp")
        for k, ct in enumerate(cts):
            nc.tensor.matmul(
                tp[:, k, :],
                lhsT=o_flat[:, qt, ct * 128:(ct + 1) * 128],
                rhs=ident,
                is_transpose=True, start=(k == 0), stop=True,
                skip_group_check=True,
            )
        nc.vector.tensor_copy(out=o_c[:, cts[0]:cts[0] + nct, qt, :], in_=tp)

    def emit_pp1(qt, fh):
        # partial out^T = o^T(ct0..4) @ w_p half + bias, parked in SBUF
        pp1 = ps_gen.tile([128, 512], F32, tag="gen", name="pp1")
        for ct in range(CT - 1):
            nc.tensor.matmul(
                pp1[:, 0:384],
                lhsT=o_c[:, ct, qt, :],
                rhs=wp_sb[:, ct, fh * 384:(fh + 1) * 384],
                start=(ct == 0), stop=(ct == CT - 2),
            )
        nc.vector.tensor_tensor(
            out=partialb[:, qt, fh, :], in0=pp1[:, 0:384],
            in1=bp_bc[:, fh * 384:(fh + 1) * 384], op=AluOpType.add)

    pending_tail = [(emit_tp, (qt, [0, 1, 2, 3, 4])) for qt in range(NQT)] + \
                   [(emit_pp1, (qt, fh)) for qt in range(NQT) for fh in range(2)]
    slot_n = [0]

    # ---------------- per head-group: kq proj + attention ----------
    wkq0 = wgp.tile([128, CT, 256], FP16, tag="wkq", name="wkq0")
    nc.gpsimd.dma_start(out=wkq0, in_=ins["wkq"][0])
    nc.gpsimd.dma_start(out=wv_sb, in_=wvv)
    for g in range(NG):
        if g == 0:
            wkq = wkq0
        else:
            wkq = wgp.tile([128, CT, 256], FP16, tag="wkq")
            nc.gpsimd.dma_start(out=wkq, in_=ins["wkq"][g])
        wkg = wkq[:, :, 0:128]
        wqg = wkq[:, :, 128:256]

        qg_sb = kqp.tile([128, QP], FP16, tag="qg")
        kgA = kqp.tile([128, QP], FP16, tag="kgA")
        kgB = kqp.tile([128, QP], FP16, tag="kgB")
        qps = ps_gen.tile([128, 512], F32, tag="gen", name="qps")
        kpsA = ps_gen.tile([128, 512], F32, tag="gen", name="kpsA")
        # q and kA interleaved per x-quarter so g0 overlaps the x DMA chunks
        for cts in ((0, 3), (3, 6)):
            for ct in range(*cts):
                nc.tensor.matmul(
                    qps[:, :], lhsT=wqg[:, ct, :], rhs=x_sb[:, ct, 0:QP],
                    start=(ct == 0), stop=(ct == CT - 1),
                )
            for ct in range(*cts):
                nc.tensor.matmul(
                    kpsA[:, :], lhsT=wkg[:, ct, :], rhs=x_sb[:, ct, 0:QP],
                    start=(ct == 0), stop=(ct == CT - 1),
                )
        nc.vector.tensor_scalar_add(qg_sb[:, :], qps, bq_sb[:, g:g + 1])
        nc.vector.tensor_scalar_add(kgA[:, :], kpsA, bk_sb[:, g:g + 1])

        def gen_kB():
            kpsB = ps_gen.tile([128, 512], F32, tag="gen", name="kpsB")
            for ct in range(CT):
                nc.tensor.matmul(
                    kpsB[:, :], lhsT=wkg[:, ct, :], rhs=x_sb[:, ct, 512:1024],
                    start=(ct == 0), stop=(ct == CT - 1),
                )
            nc.vector.tensor_scalar_add(kgB[:, :], kpsB, bk_sb[:, g:g + 1])

        if g > 0:
            gen_kB()
        # for g0, kB waits on the second x half-DMA; deferring it into h0's
        # b1 slot keeps it from gating the first scores/exps

        if g == 2:
            wpv = ins["wp"].rearrange("(t p) m -> p t m", p=128)
            nc.gpsimd.dma_start(out=wp_sb, in_=wpv)

        rc_g = smal.tile([128, NQT, 4], F32, tag="rcg")

        def emit_avs(o_ps, h, et, b):
            for i in range(2):
                kt = 2 * b + i
                for qt in range(NQT):
                    # start=True zeroes the whole 2KB bank; only the very
                    # first matmul of the head may set it
                    nc.tensor.matmul(
                        o_ps[:, qt, 0:25],
                        lhsT=et[:, i, qt * 128:(qt + 1) * 128],
                        rhs=vt_sb[:, kt, h, 0:25],
                        start=(kt == 0 and qt == 0), stop=(kt == PT - 1),
                        skip_group_check=True,
                    )

        def finish_head(o_ps, j, h):
            # denominators: column 24 of o_ps -> reciprocal -> one broadcast
            # multiply fuses division into the PSUM->SBUF move
            nc.vector.reciprocal(rc_g[:, :, j], o_ps[:, :, 24])
            nc.vector.tensor_tensor(
                out=o_sbT[:, :, h, :],
                in0=o_ps[:, :, 0:HD],
                in1=rc_g[:, :, j].unsqueeze(2).to_broadcast((128, NQT, HD)),
                op=AluOpType.mult,
            )

        deferred = []
        for j in range(4):
            h = 4 * g + j
            b0 = 32 * j
            defer = (g == 0 and j < 2)  # vt half0 still streaming during h0/h1
            if not defer:
                o_ps = ps_o.tile([128, NQT, 32], F32, tag="ops", name="o_ps")
            ets = []
            for b in range(4):  # kt pairs
                sps = ps_sps.tile([128, 2, QP], F32, tag="sps", name="sps")
                for i in range(2):
                    kt = 2 * b + i
                    ksrc = kgA if kt < 4 else kgB
                    nc.tensor.matmul(
                        sps[:, i, :],
                        lhsT=ksrc[b0:b0 + HD, (kt % 4) * 128:(kt % 4 + 1) * 128],
                        rhs=qg_sb[b0:b0 + HD, :],
                        start=True, stop=True, tile_position=(b0, 0),
                    )
                et = expp.tile([128, 2, QP], FP16, tag="exp", name="et")
                nc.scalar.activation(et[:, :, :], sps[:, :, :], Exp, scale=SCALE)
                # one vt tile per scores-slot in g0 (h0/h1); half1 paced at
                # every 4th slot across g1-g3 (g1 alone would starve the ACT)
                slot_n[0] += 1
                if g == 0 and j == 0 and b == 1:
                    gen_kB()
                if pending_vt and (defer or (g in (1, 2, 3) and slot_n[0] % 4 == 1)):
                    emit_vt_tile(*pending_vt.pop(0))
                # tail pre-work (transposes + partial proj) rides g7's slack
                if g == NG - 1 and pending_tail:
                    fn, args = pending_tail.pop(0)
                    fn(*args)
                if defer:
                    ets.append(et)
                else:
                    emit_avs(o_ps, h, et, b)
            if defer:
                deferred.append((j, h, ets))
            else:
                finish_head(o_ps, j, h)
            if g == 0 and j == 1:
                # vt half0 complete: run h0's and h1's avs now
                for dj, dh, dets in deferred:
                    o_ps = ps_o.tile([128, NQT, 32], F32, tag="ops", name="o_ps")
                    for b in range(4):
                        emit_avs(o_ps, dh, dets[b], b)
                    finish_head(o_ps, dj, dh)
                deferred = []

    if os.environ.get("KDBG", "0") == "1":
        nc.sync.dma_start(out=outs["dbg_osbt"], in_=o_sbT)
        nc.sync.dma_start(out=outs["dbg_vt"], in_=vt_sb[:, :, :, 0:25])
        nc.sync.dma_start(out=outs["dbg_rc"], in_=rc_g)

    # ---------------- tail: only the last channel chunk (ct5) remains ------
    for qt in range(NQT):
        emit_tp(qt, [5])
    outv = outs["out"].rearrange("(t p) (a b) -> t p a b", p=128, a=2)
    for qt in range(NQT):
        out_t = outp.tile([128, 2, 384], F32, tag="out")
        for fh in range(2):
            # alternate psum pools so the proj matmuls don't wait on the adds
            pool, tg = (ps_gen, "gen") if (2 * qt + fh) % 2 == 0 else (ps_o, "ops")
            pp2 = pool.tile([128, 512], F32, tag=tg, name="pp2")
            nc.tensor.matmul(
                pp2[:, 0:384],
                lhsT=o_c[:, 5, qt, :],
                rhs=wp_sb[:, 5, fh * 384:(fh + 1) * 384],
                start=True, stop=True,
            )
            nc.vector.tensor_tensor(
                out=out_t[:, fh, :], in0=pp2[:, 0:384],
                in1=partialb[:, qt, fh, :], op=AluOpType.add)
        # alternate DMA queues so the 4 output copies overlap
        eng = nc.sync if qt % 2 == 0 else nc.gpsimd
        eng.dma_start(out=outv[qt], in_=out_t)

    ctx.close()


# ------------------------- host side -------------------------

def build_inmaps(x, w_qkv, b_qkv, w_proj, b_proj):
    x = np.ascontiguousarray(x, dtype=np.float32)
    w_qkv = np.asarray(w_qkv, dtype=np.float32)
    b_qkv = np.asarray(b_qkv, dtype=np.float32)
    w_proj = np.asarray(w_proj, dtype=np.float32)
    b_proj = np.asarray(b_proj, dtype=np.float32)

    w_q, w_k, w_v = w_qkv[:, :C], w_qkv[:, C:2 * C], w_qkv[:, 2 * C:]
    b_q, b_k, b_v = b_qkv[:C], b_qkv[C:2 * C], b_qkv[2 * C:]

    def pad_w(w):  # [768, 768] -> [768, 1024] with 24->32 head col padding
        out = np.zeros((C, NH, 32), dtype=np.float32)
        out[:, :, :HD] = w.reshape(C, NH, HD)
        return out.reshape(C, NH * 32)

    def pad_b(b):  # [768] -> [128, 8]
        out = np.zeros((4, 32, NG), dtype=np.float32)
        out[:, :HD, :] = b.reshape(NG, 4, HD).transpose(1, 2, 0)
        return out.reshape(128, NG)

    wk_g = pad_w(w_k).reshape(C, NG, 128).transpose(1, 0, 2)   # [NG, C, 128]
    wq_g = pad_w(w_q).reshape(C, NG, 128).transpose(1, 0, 2)
    wkq = np.concatenate([wk_g, wq_g], axis=2)                 # [NG, C, 256]
    # preswizzle to [NG, 128, CT, 256] so each partition's DMA read is contiguous
    wkq = np.ascontiguousarray(
        wkq.reshape(NG, CT, 128, 256).transpose(0, 2, 1, 3)).astype(np.float16)
    bk = pad_b(b_k)
    bq = pad_b(b_q)
    # b_v folded into the proj bias (attention weights sum to 1)
    bp1 = (b_proj + w_proj.T @ b_v).astype(np.float32)
    ident = np.eye(128, dtype=np.float16)

    in_maps = []
    for core in range(8):
        b, half = core // 2, core % 2
        xb = x[b].reshape(C, HW)
        # rotate so this core's queries are always columns 0:QP (keys are
        # permutation-invariant under softmax)
        xb = np.ascontiguousarray(np.roll(xb, -half * QP, axis=1)).astype(np.float16)
        in_maps.append({
            "x": xb,
            "wkq": wkq,
            "wv": np.ascontiguousarray(w_v).astype(np.float16),
            "wp": np.ascontiguousarray(w_proj).astype(np.float16),
            "bk": bk, "bq": bq, "bp1": bp1,
            "ident": ident,
        })
    return in_maps


_PROGRAM = None


def build_program():
    global _PROGRAM
    if _PROGRAM is not None:
        return _PROGRAM
    nc = bacc.Bacc("TRN2", target_bir_lowering=False, debug=False)
    ins = {
        "x": nc.dram_tensor("x", [C, HW], FP16, kind="ExternalInput").ap(),
        "wkq": nc.dram_tensor("wkq", [NG, 128, CT, 256], FP16, kind="ExternalInput").ap(),
        "wv": nc.dram_tensor("wv", [C, C], FP16, kind="ExternalInput").ap(),
        "wp": nc.dram_tensor("wp", [C, C], FP16, kind="ExternalInput").ap(),
        "bk": nc.dram_tensor("bk", [128, NG], F32, kind="ExternalInput").ap(),
        "bq": nc.dram_tensor("bq", [128, NG], F32, kind="ExternalInput").ap(),
        "bp1": nc.dram_tensor("bp1", [C], F32, kind="ExternalInput").ap(),
        "ident": nc.dram_tensor("ident", [128, 128], FP16, kind="ExternalInput").ap(),
    }
    outs = {"out": nc.dram_tensor("out", [QP, C], F32, kind="ExternalOutput").ap()}
    if os.environ.get("KDBG", "0") == "1":
        outs["dbg_osbt"] = nc.dram_tensor(
            "dbg_osbt", [128, NQT, NH, HD], FP16, kind="ExternalOutput").ap()
        outs["dbg_vt"] = nc.dram_tensor(
            "dbg_vt", [128, PT, NH, 25], FP16, kind="ExternalOutput").ap()
        outs["dbg_rc"] = nc.dram_tensor(
            "dbg_rc", [128, NQT, 4], F32, kind="ExternalOutput").ap()
    with tile.TileContext(nc) as tc:
        emit_kernel(tc, outs, ins)
    nc.compile()
    _PROGRAM = nc
    return nc


def run(inputs, trace=False):
    nc = build_program()
    in_maps = build_inmaps(**inputs)
    try:
        res = bass_utils.run_bass_kernel_spmd(
            nc, in_maps, core_ids=list(range(8)), trace=trace)
    except ModuleNotFoundError:
        # BASS_TRACE path needs antenv.axon_hooks, absent in some containers;
        # rerun untraced rather than failing.
        prev = os.environ.get("BASS_NEVER_TRACE")
        os.environ["BASS_NEVER_TRACE"] = "1"
        try:
            res = bass_utils.run_bass_kernel_spmd(
                nc, in_maps, core_ids=list(range(8)), trace=False)
        finally:
            if prev is None:
                os.environ.pop("BASS_NEVER_TRACE", None)
            else:
                os.environ["BASS_NEVER_TRACE"] = prev
    out_full = np.empty((4, C, HW), dtype=np.float32)
    for core in range(8):
        b, half = core // 2, core % 2
        out_full[b][:, half * QP:(half + 1) * QP] = res.results[core]["out"].T
    return out_full.reshape(4, C, 32, 32), res


def kernel(**inputs):
    out, _ = run(inputs, trace=False)
    return out


# revision 38
# speedup vs baseline: 1.0020x; 1.0020x over previous
"""Attention2d SPMD kernel for 8 TRN2 NeuronCores.

Problem (hardcoded): x [4, 768, 32, 32], w_qkv [768, 2304], b_qkv [2304],
w_proj [768, 768], b_proj [768]; 32 heads, head_dim 24.

Sharding: 8 cores = 4 batches x 2 query-halves (512 queries each).
Each core computes k/v for all 1024 positions of its batch (2x duplicated
across the pair of cores sharing a batch) and q/attention/proj for its own
512 query positions. Outputs are disjoint slices -> host gather is pure
concatenation (no collectives). Per-core x is ROTATED on the host so each
core's queries are always columns 0:512 (softmax is permutation-invariant
over keys), which makes the SPMD program identical across cores.

Per-core dataflow (per head-group g of 4 heads):
  k_g = w_k^T x  [128ch_pad, 1024]  (fp16)     q_g = w_q^T x  [128, 512]
  vT  = x^T w_v  [1024pos, 32 heads x (24ch | ones-col | 7 pad)]  (fp16)
  per head h, kt in 8 key-tiles: sT = k_h^T q_h [128k, 512q] -> Exp ->
    oT[128q-tile, 25] += et[:, qt]^T vT_h    (TRANSPOSED attn@v: queries on
    PSUM partitions, head_dim on the free axis -> 25-cycle matmuls; the
    vT ones-column lands the softmax denominator in oT column 24)
  divide: oT[:, 0:24] * (1/denom col) via one broadcast tensor_tensor per
    head (denominator is a per-partition column now - no DRAM bounce)
  tail: PE-transpose oT -> o [c, q] (identity matmul), then
    out^T[q, 768] = o^T W_p + b_p'   with b_p' = b_proj + W_p^T b_v folded
    on the host (exact: attention weights sum to 1). Host transposes out^T.

Precision: fp16 operands everywhere on the PE (1 cyc/row), fp32 PSUM,
denominator division exact fp32.
"""

import os
import numpy as np

import concourse.bacc as bacc
import concourse.bass as bass
import concourse.mybir as mybir
import concourse.tile as tile
from concourse import bass_utils
from concourse.alu_op_type import AluOpType

C = 768
HW = 1024
QP = 512          # queries per core
NH = 32           # heads
HD = 24           # head dim
NG = 8            # head groups (4 heads each, 32-padded rows)
CT = C // 128     # 6 contraction tiles
PT = HW // 128    # 8 position tiles
NQT = QP // 128   # 4 query tiles
SCALE = HD ** -0.5
BF16 = mybir.dt.bfloat16
FP16 = mybir.dt.float16
F32 = mybir.dt.float32


def emit_kernel(tc, outs, ins):
    from contextlib import ExitStack
    nc = tc.nc
    ctx = ExitStack()
    Exp = mybir.ActivationFunctionType.Exp

    big = ctx.enter_context(tc.tile_pool(name="big", bufs=1))
    kqp = ctx.enter_context(tc.tile_pool(name="kqp", bufs=2))
    wgp = ctx.enter_context(tc.tile_pool(name="wgp", bufs=3))
    expp = ctx.enter_context(tc.tile_pool(name="expp", bufs=8))
    smal = ctx.enter_context(tc.tile_pool(name="smal", bufs=2))
    outp = ctx.enter_context(tc.tile_pool(name="outp", bufs=4))
    # PSUM budget (8 banks): sps 2x[128,2,512]=4, gen 2x[128,512]=2,
    # oT 2x[128,4,32]=2.  Tail transpose/proj tiles reuse the sps slots.
    ps_sps = ctx.enter_context(tc.tile_pool(name="ps_sps", bufs=2, space="PSUM"))
    ps_gen = ctx.enter_context(tc.tile_pool(name="ps_gen", bufs=2, space="PSUM"))
    ps_o = ctx.enter_context(tc.tile_pool(name="ps_o", bufs=2, space="PSUM"))

    # ---------------- persistent SBUF tensors ----------------
    x_sb = big.tile([128, CT, HW], FP16)
    wv_sb = big.tile([128, CT, C], FP16)
    wp_sb = big.tile([128, CT, C], FP16)           # w_proj [c,f], c-chunked
    vt_sb = big.tile([128, PT, NH, 32], FP16)      # 2 MB; col HD is ones
    o_sbT = big.tile([128, NQT, NH, HD], FP16)     # divided o^T
    o_c = big.tile([128, CT, NQT, 128], FP16)      # transposed o (c on part)
    bk_sb = big.tile([128, NG], F32)
    bq_sb = big.tile([128, NG], F32)
    bp_bc = big.tile([128, C], F32)                # b_proj' bcast to all part
    ident = big.tile([128, 128], FP16)

    # DMA queues: SP carries ident + x (2 column-half DMAs: q/kA only need
    # cols 0:512, so the PE can start ~2.4us earlier) + the small tensors;
    # Pool carries the weight streams.  One DMA per tensor: each dma_start
    # pays ~1us of SWDGE fixed cost, so per-chunk DMAs serialize the start.
    xv = ins["x"].rearrange("(t p) n -> p t n", p=128)
    wvv = ins["wv"].rearrange("(t p) m -> p t m", p=128)
    warm_sb = big.tile([1, 2], F32)
    nc.vector.memset(warm_sb, 0.0)
    nc.scalar.activation(warm_sb[:, 1:2], warm_sb[:, 0:1], Exp, scale=1.0)
    nc.sync.dma_start(out=ident, in_=ins["ident"])
    nc.sync.dma_start(out=x_sb[:, 0:3, 0:512], in_=xv[:, 0:3, 0:512])
    nc.sync.dma_start(out=x_sb[:, 3:6, 0:512], in_=xv[:, 3:6, 0:512])
    nc.sync.dma_start(out=x_sb[:, :, 512:1024], in_=xv[:, :, 512:1024])
    nc.scalar.dma_start(out=bk_sb, in_=ins["bk"])
    nc.scalar.dma_start(out=bq_sb, in_=ins["bq"])
    nc.scalar.dma_start(out=bp_bc, in_=ins["bp1"].unsqueeze(0).to_broadcast((128, C)))
    # only vt column 24 (the denominator ones-column) is ever read beyond 0:24
    nc.vector.memset(vt_sb[:, :, :, 24:25], 1.0)
    # keep the PE continuously busy from ~t=2.5us so its p-state ramp
    # completes before the first real matmul
    warm_ps = ps_o.tile([128, 128], F32, tag="ops", name="warm_ps")
    for _ in range(30):
        nc.tensor.matmul(warm_ps, lhsT=ident, rhs=ident,
                         start=True, stop=True, skip_group_check=True)

    def emit_vt_tile(t, pt):
        # vT for heads 16t..16t+16 (dense, N=384) at position tile pt
        vps = ps_gen.tile([128, 512], F32, tag="gen", name="vps")
        for ct in range(CT):
            nc.tensor.matmul(
                vps[:, 0:384],
                lhsT=x_sb[:, ct, pt * 128:(pt + 1) * 128],
                rhs=wv_sb[:, ct, 384 * t:384 * (t + 1)],
                start=(ct == 0), stop=(ct == CT - 1),
            )
        nc.vector.tensor_copy(
            out=vt_sb[:, pt, 16 * t:16 * (t + 1), 0:HD],
            in_=vps[:, 0:384].rearrange("p (h d) -> p h d", d=HD),
        )

    # vT tiles pending emission: one per scores-slot during g0/g1 so the
    # PE never bursts 2+ vt tiles between exps (which would starve the ACT)
    pending_vt = [(0, pt) for pt in range(PT)] + [(1, pt) for pt in range(PT)]

    o_flat = o_sbT.rearrange("p a h d -> p a (h d)")
    partialb = big.tile([128, NQT, 2, 384], F32)   # proj(ct0..4) + bias

    def emit_tp(qt, cts):
        # PE-transpose o^T chunks -> o_c (c on partitions)
        nct = len(cts)
        tp = ps_gen.tile([128, nct, 128], FP16, tag="gen", name="tp")
        for k, ct in enumerate(cts):
            nc.tensor.matmul(
                tp[:, k, :],
                lhsT=o_flat[:, qt, ct * 128:(ct + 1) * 128],
                rhs=ident,
                is_transpose=True, start=(k == 0), stop=True,
                skip_group_check=True,
            )
        nc.vector.tensor_copy(out=o_c[:, cts[0]:cts[0] + nct, qt, :], in_=tp)

    def emit_pp1(qt, fh):
        # partial out^T = o^T(ct0..4) @ w_p half + bias, parked in SBUF
        pp1 = ps_gen.tile([128, 512], F32, tag="gen", name="pp1")
        for ct in range(CT - 1):
            nc.tensor.matmul(
                pp1[:, 0:384],
                lhsT=o_c[:, ct, qt, :],
                rhs=wp_sb[:, ct, fh * 384:(fh + 1) * 384],
                start=(ct == 0), stop=(ct == CT - 2),
            )
        nc.vector.tensor_tensor(
            out=partialb[:, qt, fh, :], in0=pp1[:, 0:384],
            in1=bp_bc[:, fh * 384:(fh + 1) * 384], op=AluOpType.add)

    pending_tail = [(emit_tp, (qt, [0, 1, 2, 3, 4])) for qt in range(NQT)] + \
                   [(emit_pp1, (qt, fh)) for qt in range(NQT) for fh in range(2)]
    slot_n = [0]

    # ---------------- per head-group: kq proj + attention ----------
    wkq0 = wgp.tile([128, CT, 256], FP16, tag="wkq", name="wkq0")
    nc.gpsimd.dma_start(out=wkq0, in_=ins["wkq"][0])
    nc.gpsimd.dma_start(out=wv_sb, in_=wvv)
    for g in range(NG):
        if g == 0:
            wkq = wkq0
        else:
            wkq = wgp.tile([128, CT, 256], FP16, tag="wkq")
            nc.gpsimd.dma_start(out=wkq, in_=ins["wkq"][g])
        wkg = wkq[:, :, 0:128]
        wqg = wkq[:, :, 128:256]

        qg_sb = kqp.tile([128, QP], FP16, tag="qg")
        kgA = kqp.tile([128, QP], FP16, tag="kgA")
        kgB = kqp.tile([128, QP], FP16, tag="kgB")
        qps = ps_gen.tile([128, 512], F32, tag="gen", name="qps")
        kpsA = ps_gen.tile([128, 512], F32, tag="gen", name="kpsA")
        # q and kA interleaved per x-quarter so g0 overlaps the x DMA chunks
        for cts in ((0, 3), (3, 6)):
            for ct in range(*cts):
                nc.tensor.matmul(
                    qps[:, :], lhsT=wqg[:, ct, :], rhs=x_sb[:, ct, 0:QP],
                    start=(ct == 0), stop=(ct == CT - 1),
                )
            for ct in range(*cts):
                nc.tensor.matmul(
                    kpsA[:, :], lhsT=wkg[:, ct, :], rhs=x_sb[:, ct, 0:QP],
                    start=(ct == 0), stop=(ct == CT - 1),
                )
        nc.vector.tensor_scalar_add(qg_sb[:, :], qps, bq_sb[:, g:g + 1])
        nc.vector.tensor_scalar_add(kgA[:, :], kpsA, bk_sb[:, g:g + 1])

        def gen_kB():
            kpsB = ps_gen.tile([128, 512], F32, tag="gen", name="kpsB")
            for ct in range(CT):
                nc.tensor.matmul(
                    kpsB[:, :], lhsT=wkg[:, ct, :], rhs=x_sb[:, ct, 512:1024],
                    start=(ct == 0), stop=(ct == CT - 1),
                )
            nc.vector.tensor_scalar_add(kgB[:, :], kpsB, bk_sb[:, g:g + 1])

        if g > 0:
            gen_kB()
        # for g0, kB waits on the second x half-DMA; deferring it into h0's
        # b1 slot keeps it from gating the first scores/exps

        if g == 2:
            wpv = ins["wp"].rearrange("(t p) m -> p t m", p=128)
            nc.gpsimd.dma_start(out=wp_sb, in_=wpv)

        rc_g = smal.tile([128, NQT, 4], F32, tag="rcg")

        def emit_avs(o_ps, h, et, b):
            for i in range(2):
                kt = 2 * b + i
                for qt in range(NQT):
                    # start=True zeroes the whole 2KB bank; only the very
                    # first matmul of the head may set it
                    nc.tensor.matmul(
                        o_ps[:, qt, 0:25],
                        lhsT=et[:, i, qt * 128:(qt + 1) * 128],
                        rhs=vt_sb[:, kt, h, 0:25],
                        start=(kt == 0 and qt == 0), stop=(kt == PT - 1),
                        skip_group_check=True,
                    )

        def finish_head(o_ps, j, h):
            # denominators: column 24 of o_ps -> reciprocal -> one broadcast
            # multiply fuses division into the PSUM->SBUF move
            nc.vector.reciprocal(rc_g[:, :, j], o_ps[:, :, 24])
            nc.vector.tensor_tensor(
                out=o_sbT[:, :, h, :],
                in0=o_ps[:, :, 0:HD],
                in1=rc_g[:, :, j].unsqueeze(2).to_broadcast((128, NQT, HD)),
                op=AluOpType.mult,
            )

        deferred = []
        for j in range(4):
            h = 4 * g + j
            b0 = 32 * j
            defer = (g == 0 and j < 2)  # vt half0 still streaming during h0/h1
            if not defer:
                o_ps = ps_o.tile([128, NQT, 32], F32, tag="ops", name="o_ps")
            ets = []
            for b in range(4):  # kt pairs
                sps = ps_sps.tile([128, 2, QP], F32, tag="sps", name="sps")
                for i in range(2):
                    kt = 2 * b + i
                    ksrc = kgA if kt < 4 else kgB
                    nc.tensor.matmul(
                        sps[:, i, :],
                        lhsT=ksrc[b0:b0 + HD, (kt % 4) * 128:(kt % 4 + 1) * 128],
                        rhs=qg_sb[b0:b0 + HD, :],
                        start=True, stop=True, tile_position=(b0, 0),
                    )
                et = expp.tile([128, 2, QP], FP16, tag="exp", name="et")
                nc.scalar.activation(et[:, :, :], sps[:, :, :], Exp, scale=SCALE)
                # one vt tile per scores-slot in g0 (h0/h1); half1 paced at
                # every 4th slot across g1-g3 (g1 alone would starve the ACT)
                slot_n[0] += 1
                if g == 0 and j == 0 and b == 1:
                    gen_kB()
                if pending_vt and (defer or (g in (1, 2, 3) and slot_n[0] % 4 == 1)):
                    emit_vt_tile(*pending_vt.pop(0))
                # tail pre-work (transposes + partial proj) rides g7's slack
                if g == NG - 1 and pending_tail:
                    fn, args = pending_tail.pop(0)
                    fn(*args)
                if defer:
                    ets.append(et)
                else:
                    emit_avs(o_ps, h, et, b)
            if defer:
                deferred.append((j, h, ets))
            else:
                finish_head(o_ps, j, h)
            if g == 0 and j == 1:
                # vt half0 complete: run h0's and h1's avs now
                for dj, dh, dets in deferred:
                    o_ps = ps_o.tile([128, NQT, 32], F32, tag="ops", name="o_ps")
                    for b in range(4):
                        emit_avs(o_ps, dh, dets[b], b)
                    finish_head(o_ps, dj, dh)
                deferred = []

    if os.environ.get("KDBG", "0") == "1":
        nc.sync.dma_start(out=outs["dbg_osbt"], in_=o_sbT)
        nc.sync.dma_start(out=outs["dbg_vt"], in_=vt_sb[:, :, :, 0:25])
        nc.sync.dma_start(out=outs["dbg_rc"], in_=rc_g)

    # ---------------- tail: only the last channel chunk (ct5) remains ------
    for qt in range(NQT):
        emit_tp(qt, [5])
    outv = outs["out"].rearrange("(t p) (a b) -> t p a b", p=128, a=2)
    for qt in range(NQT):
        out_t = outp.tile([128, 2, 384], F32, tag="out")
        for fh in range(2):
            # alternate psum pools so the proj matmuls don't wait on the adds
            pool, tg = (ps_gen, "gen") if (2 * qt + fh) % 2 == 0 else (ps_o, "ops")
            pp2 = pool.tile([128, 512], F32, tag=tg, name="pp2")
            nc.tensor.matmul(
                pp2[:, 0:384],
                lhsT=o_c[:, 5, qt, :],
                rhs=wp_sb[:, 5, fh * 384:(fh + 1) * 384],
                start=True, stop=True,
            )
            nc.vector.tensor_tensor(
                out=out_t[:, fh, :], in0=pp2[:, 0:384],
                in1=partialb[:, qt, fh, :], op=AluOpType.add)
        # alternate DMA queues so the 4 output copies overlap
        eng = nc.sync if qt % 2 == 0 else nc.gpsimd
        eng.dma_start(out=outv[qt], in_=out_t)

    ctx.close()


# ------------------------- host side -------------------------

def build_inmaps(x, w_qkv, b_qkv, w_proj, b_proj):
    x = np.ascontiguousarray(x, dtype=np.float32)
    w_qkv = np.asarray(w_qkv, dtype=np.float32)
    b_qkv = np.asarray(b_qkv, dtype=np.float32)
    w_proj = np.asarray(w_proj, dtype=np.float32)
    b_proj = np.asarray(b_proj, dtype=np.float32)

    w_q, w_k, w_v = w_qkv[:, :C], w_qkv[:, C:2 * C], w_qkv[:, 2 * C:]
    b_q, b_k, b_v = b_qkv[:C], b_qkv[C:2 * C], b_qkv[2 * C:]

    def pad_w(w):  # [768, 768] -> [768, 1024] with 24->32 head col padding
        out = np.zeros((C, NH, 32), dtype=np.float32)
        out[:, :, :HD] = w.reshape(C, NH, HD)
        return out.reshape(C, NH * 32)

    def pad_b(b):  # [768] -> [128, 8]
        out = np.zeros((4, 32, NG), dtype=np.float32)
        out[:, :HD, :] = b.reshape(NG, 4, HD).transpose(1, 2, 0)
        return out.reshape(128, NG)

    wk_g = pad_w(w_k).reshape(C, NG, 128).transpose(1, 0, 2)   # [NG, C, 128]
    wq_g = pad_w(w_q).reshape(C, NG, 128).transpose(1, 0, 2)
    wkq = np.concatenate([wk_g, wq_g], axis=2)                 # [NG, C, 256]
    # preswizzle to [NG, 128, CT, 256] so each partition's DMA read is contiguous
    wkq = np.ascontiguousarray(
        wkq.reshape(NG, CT, 128, 256).transpose(0, 2, 1, 3)).astype(np.float16)
    bk = pad_b(b_k)
    bq = pad_b(b_q)
    # b_v folded into the proj bias (attention weights sum to 1)
    bp1 = (b_proj + w_proj.T @ b_v).astype(np.float32)
    ident = np.eye(128, dtype=np.float16)

    in_maps = []
    for core in range(8):
        b, half = core // 2, core % 2
        xb = x[b].reshape(C, HW)
        # rotate so this core's queries are always columns 0:QP (keys are
        # permutation-invariant under softmax)
        xb = np.ascontiguousarray(np.roll(xb, -half * QP, axis=1)).astype(np.float16)
        in_maps.append({
            "x": xb,
            "wkq": wkq,
            "wv": np.ascontiguousarray(w_v).astype(np.float16),
            "wp": np.ascontiguousarray(w_proj).astype(np.float16),
            "bk": bk, "bq": bq, "bp1": bp1,
            "ident": ident,
        })
    return in_maps


_PROGRAM = None


def build_program():
    global _PROGRAM
    if _PROGRAM is not None:
        return _PROGRAM
    nc = bacc.Bacc("TRN2", target_bir_lowering=False, debug=False)
    ins = {
        "x": nc.dram_tensor("x", [C, HW], FP16, kind="ExternalInput").ap(),
        "wkq": nc.dram_tensor("wkq", [NG, 128, CT, 256], FP16, kind="ExternalInput").ap(),
        "wv": nc.dram_tensor("wv", [C, C], FP16, kind="ExternalInput").ap(),
        "wp": nc.dram_tensor("wp", [C, C], FP16, kind="ExternalInput").ap(),
        "bk": nc.dram_tensor("bk", [128, NG], F32, kind="ExternalInput").ap(),
        "bq": nc.dram_tensor("bq", [128, NG], F32, kind="ExternalInput").ap(),
        "bp1": nc.dram_tensor("bp1", [C], F32, kind="ExternalInput").ap(),
        "ident": nc.dram_tensor("ident", [128, 128], FP16, kind="ExternalInput").ap(),
    }
    outs = {"out": nc.dram_tensor("out", [QP, C], F32, kind="ExternalOutput").ap()}
    if os.environ.get("KDBG", "0") == "1":
        outs["dbg_osbt"] = nc.dram_tensor(
            "dbg_osbt", [128, NQT, NH, HD], FP16, kind="ExternalOutput").ap()
        outs["dbg_vt"] = nc.dram_tensor(
            "dbg_vt", [128, PT, NH, 25], FP16, kind="ExternalOutput").ap()
        outs["dbg_rc"] = nc.dram_tensor(
            "dbg_rc", [128, NQT, 4], F32, kind="ExternalOutput").ap()
    with tile.TileContext(nc) as tc:
        emit_kernel(tc, outs, ins)
    nc.compile()
    _PROGRAM = nc
    return nc


def run(inputs, trace=False):
    nc = build_program()
    in_maps = build_inmaps(**inputs)
    try:
        res = bass_utils.run_bass_kernel_spmd(
            nc, in_maps, core_ids=list(range(8)), trace=trace)
    except ModuleNotFoundError:
        # BASS_TRACE path needs antenv.axon_hooks, absent in some containers;
        # rerun untraced rather than failing.
        prev = os.environ.get("BASS_NEVER_TRACE")
        os.environ["BASS_NEVER_TRACE"] = "1"
        try:
            res = bass_utils.run_bass_kernel_spmd(
                nc, in_maps, core_ids=list(range(8)), trace=False)
        finally:
            if prev is None:
                os.environ.pop("BASS_NEVER_TRACE", None)
            else:
                os.environ["BASS_NEVER_TRACE"] = prev
    out_full = np.empty((4, C, HW), dtype=np.float32)
    for core in range(8):
        b, half = core // 2, core % 2
        out_full[b][:, half * QP:(half + 1) * QP] = res.results[core]["out"].T
    return out_full.reshape(4, C, 32, 32), res


def kernel(**inputs):
    out, _ = run(inputs, trace=False)
    return out


# revision 42
# speedup vs baseline: 1.0230x; 1.0210x over previous
"""Attention2d SPMD kernel for 8 TRN2 NeuronCores.

Problem (hardcoded): x [4, 768, 32, 32], w_qkv [768, 2304], b_qkv [2304],
w_proj [768, 768], b_proj [768]; 32 heads, head_dim 24.

Sharding: 8 cores = 4 batches x 2 query-halves (512 queries each).
Each core computes k/v for all 1024 positions of its batch (2x duplicated
across the pair of cores sharing a batch) and q/attention/proj for its own
512 query positions. Outputs are disjoint slices -> host gather is pure
concatenation (no collectives). Per-core x is ROTATED on the host so each
core's queries are always columns 0:512 (softmax is permutation-invariant
over keys), which makes the SPMD program identical across cores.

Per-core dataflow (per head-group g of 4 heads):
  k_g = w_k^T x  [128ch_pad, 1024]  (fp16)     q_g = w_q^T x  [128, 512]
  vT  = x^T w_v  [1024pos, 32 heads x (24ch | ones-col | 7 pad)]  (fp16)
  per head h, kt in 8 key-tiles: sT = k_h^T q_h [128k, 512q] -> Exp ->
    oT[128q-tile, 25] += et[:, qt]^T vT_h    (TRANSPOSED attn@v: queries on
    PSUM partitions, head_dim on the free axis -> 25-cycle matmuls; the
    vT ones-column lands the softmax denominator in oT column 24)
  divide: oT[:, 0:24] * (1/denom col) via one broadcast tensor_tensor per
    head (denominator is a per-partition column now - no DRAM bounce)
  tail: PE-transpose oT -> o [c, q] (identity matmul), then
    out^T[q, 768] = o^T W_p + b_p'   with b_p' = b_proj + W_p^T b_v folded
    on the host (exact: attention weights sum to 1). Host transposes out^T.

Precision: fp16 operands everywhere on the PE (1 cyc/row), fp32 PSUM,
denominator division exact fp32.
"""

import os
import numpy as np

import concourse.bacc as bacc
import concourse.bass as bass
import concourse.mybir as mybir
import concourse.tile as tile
from concourse import bass_utils
from concourse.alu_op_type import AluOpType

C = 768
HW = 1024
QP = 512          # queries per core
NH = 32           # heads
HD = 24           # head dim
NG = 8            # head groups (4 heads each, 32-padded rows)
CT = C // 128     # 6 contraction tiles
PT = HW // 128    # 8 position tiles
NQT = QP // 128   # 4 query tiles
SCALE = HD ** -0.5
BF16 = mybir.dt.bfloat16
FP16 = mybir.dt.float16
F32 = mybir.dt.float32


def emit_kernel(tc, outs, ins):
    from contextlib import ExitStack
    nc = tc.nc
    ctx = ExitStack()
    Exp = mybir.ActivationFunctionType.Exp

    big = ctx.enter_context(tc.tile_pool(name="big", bufs=1))
    kqp = ctx.enter_context(tc.tile_pool(name="kqp", bufs=2))
    wgp = ctx.enter_context(tc.tile_pool(name="wgp", bufs=3))
    expp = ctx.enter_context(tc.tile_pool(name="expp", bufs=8))
    smal = ctx.enter_context(tc.tile_pool(name="smal", bufs=2))
    outp = ctx.enter_context(tc.tile_pool(name="outp", bufs=4))
    # PSUM budget (8 banks): sps 2x[128,2,512]=4, gen 2x[128,512]=2,
    # oT 2x[128,4,32]=2.  Tail transpose/proj tiles reuse the sps slots.
    ps_sps = ctx.enter_context(tc.tile_pool(name="ps_sps", bufs=2, space="PSUM"))
    ps_gen = ctx.enter_context(tc.tile_pool(name="ps_gen", bufs=2, space="PSUM"))
    ps_o = ctx.enter_context(tc.tile_pool(name="ps_o", bufs=2, space="PSUM"))

    # ---------------- persistent SBUF tensors ----------------
    x_sb = big.tile([128, CT, HW], FP16)
    wv_sb = big.tile([128, CT, C], FP16)
    wp_sb = big.tile([128, CT, C], FP16)           # w_proj [c,f], c-chunked
    vt_sb = big.tile([128, PT, NH, 32], FP16)      # 2 MB; col HD is ones
    o_sbT = big.tile([128, NQT, NH, HD], FP16)     # divided o^T
    o_c = big.tile([128, CT, NQT, 128], FP16)      # transposed o (c on part)
    bk_sb = big.tile([128, NG], F32)
    bq_sb = big.tile([128, NG], F32)
    bp_bc = big.tile([128, C], F32)                # b_proj' bcast to all part
    ident = big.tile([128, 128], FP16)

    # DMA queues: SP carries ident + x (2 column-half DMAs: q/kA only need
    # cols 0:512, so the PE can start ~2.4us earlier) + the small tensors;
    # Pool carries the weight streams.  One DMA per tensor: each dma_start
    # pays ~1us of SWDGE fixed cost, so per-chunk DMAs serialize the start.
    xv = ins["x"].rearrange("(t p) n -> p t n", p=128)
    wvv = ins["wv"].rearrange("(t p) m -> p t m", p=128)
    warm_sb = big.tile([1, 2], F32)
    nc.vector.memset(warm_sb, 0.0)
    nc.scalar.activation(warm_sb[:, 1:2], warm_sb[:, 0:1], Exp, scale=1.0)
    nc.sync.dma_start(out=ident, in_=ins["ident"])
    nc.sync.dma_start(out=x_sb[:, :, 0:512], in_=xv[:, :, 0:512])
    nc.sync.dma_start(out=x_sb[:, :, 512:1024], in_=xv[:, :, 512:1024])
    nc.scalar.dma_start(out=bk_sb, in_=ins["bk"])
    nc.scalar.dma_start(out=bq_sb, in_=ins["bq"])
    nc.scalar.dma_start(out=bp_bc, in_=ins["bp1"].unsqueeze(0).to_broadcast((128, C)))
    # only vt column 24 (the denominator ones-column) is ever read beyond 0:24
    nc.vector.memset(vt_sb[:, :, :, 24:25], 1.0)
    # keep the PE continuously busy from ~t=2.5us so its p-state ramp
    # completes before the first real matmul
    warm_ps = ps_o.tile([128, 128], F32, tag="ops", name="warm_ps")
    for _ in range(30):
        nc.tensor.matmul(warm_ps, lhsT=ident, rhs=ident,
                         start=True, stop=True, skip_group_check=True)

    def emit_vt_tile(t, pt):
        # vT for heads 16t..16t+16 (dense, N=384) at position tile pt
        vps = ps_gen.tile([128, 512], F32, tag="gen", name="vps")
        for ct in range(CT):
            nc.tensor.matmul(
                vps[:, 0:384],
                lhsT=x_sb[:, ct, pt * 128:(pt + 1) * 128],
                rhs=wv_sb[:, ct, 384 * t:384 * (t + 1)],
                start=(ct == 0), stop=(ct == CT - 1),
            )
        nc.vector.tensor_copy(
            out=vt_sb[:, pt, 16 * t:16 * (t + 1), 0:HD],
            in_=vps[:, 0:384].rearrange("p (h d) -> p h d", d=HD),
        )

    # vT tiles pending emission: one per scores-slot during g0/g1 so the
    # PE never bursts 2+ vt tiles between exps (which would starve the ACT)
    pending_vt = [(0, pt) for pt in range(PT)] + [(1, pt) for pt in range(PT)]

    o_flat = o_sbT.rearrange("p a h d -> p a (h d)")
    partialb = big.tile([128, NQT, 2, 384], F32)   # proj(ct0..4) + bias

    def emit_tp(qt, cts):
        # PE-transpose o^T chunks -> o_c (c on partitions)
        nct = len(cts)
        tp = ps_gen.tile([128, nct, 128], FP16, tag="gen", name="tp")
        for k, ct in enumerate(cts):
            nc.tensor.matmul(
                tp[:, k, :],
                lhsT=o_flat[:, qt, ct * 128:(ct + 1) * 128],
                rhs=ident,
                is_transpose=True, start=(k == 0), stop=True,
                skip_group_check=True,
            )
        nc.vector.tensor_copy(out=o_c[:, cts[0]:cts[0] + nct, qt, :], in_=tp)

    def emit_pp1(qt, fh):
        # partial out^T = o^T(ct0..4) @ w_p half + bias, parked in SBUF
        pp1 = ps_gen.tile([128, 512], F32, tag="gen", name="pp1")
        for ct in range(CT - 1):
            nc.tensor.matmul(
                pp1[:, 0:384],
                lhsT=o_c[:, ct, qt, :],
                rhs=wp_sb[:, ct, fh * 384:(fh + 1) * 384],
                start=(ct == 0), stop=(ct == CT - 2),
            )
        nc.vector.tensor_tensor(
            out=partialb[:, qt, fh, :], in0=pp1[:, 0:384],
            in1=bp_bc[:, fh * 384:(fh + 1) * 384], op=AluOpType.add)

    pending_tail = [(emit_tp, (qt, [0, 1, 2, 3, 4])) for qt in range(NQT)] + \
                   [(emit_pp1, (qt, fh)) for qt in range(NQT) for fh in range(2)]
    slot_n = [0]

    # ---------------- per head-group: kq proj + attention ----------
    wkq0 = wgp.tile([128, CT, 256], FP16, tag="wkq", name="wkq0")
    nc.gpsimd.dma_start(out=wkq0, in_=ins["wkq"][0])
    nc.gpsimd.dma_start(out=wv_sb, in_=wvv)
    for g in range(NG):
        if g == 0:
            wkq = wkq0
        else:
            wkq = wgp.tile([128, CT, 256], FP16, tag="wkq")
            nc.gpsimd.dma_start(out=wkq, in_=ins["wkq"][g])
        wkg = wkq[:, :, 0:128]
        wqg = wkq[:, :, 128:256]

        qg_sb = kqp.tile([128, QP], FP16, tag="qg")
        kgA = kqp.tile([128, QP], FP16, tag="kgA")
        kgB = kqp.tile([128, QP], FP16, tag="kgB")
        qps = ps_gen.tile([128, 512], F32, tag="gen", name="qps")
        kpsA = ps_gen.tile([128, 512], F32, tag="gen", name="kpsA")
        # q and kA interleaved per x-quarter so g0 overlaps the x DMA chunks
        for cts in ((0, 3), (3, 6)):
            for ct in range(*cts):
                nc.tensor.matmul(
                    qps[:, :], lhsT=wqg[:, ct, :], rhs=x_sb[:, ct, 0:QP],
                    start=(ct == 0), stop=(ct == CT - 1),
                )
            for ct in range(*cts):
                nc.tensor.matmul(
                    kpsA[:, :], lhsT=wkg[:, ct, :], rhs=x_sb[:, ct, 0:QP],
                    start=(ct == 0), stop=(ct == CT - 1),
                )
        nc.vector.tensor_scalar_add(qg_sb[:, :], qps, bq_sb[:, g:g + 1])
        nc.vector.tensor_scalar_add(kgA[:, :], kpsA, bk_sb[:, g:g + 1])

        def gen_kB():
            kpsB = ps_gen.tile([128, 512], F32, tag="gen", name="kpsB")
            for ct in range(CT):
                nc.tensor.matmul(
                    kpsB[:, :], lhsT=wkg[:, ct, :], rhs=x_sb[:, ct, 512:1024],
                    start=(ct == 0), stop=(ct == CT - 1),
                )
            nc.vector.tensor_scalar_add(kgB[:, :], kpsB, bk_sb[:, g:g + 1])

        if g > 0:
            gen_kB()
        # for g0, kB waits on the second x half-DMA; deferring it into h0's
        # b1 slot keeps it from gating the first scores/exps

        if g == 2:
            wpv = ins["wp"].rearrange("(t p) m -> p t m", p=128)
            nc.gpsimd.dma_start(out=wp_sb, in_=wpv)

        rc_g = smal.tile([128, NQT, 4], F32, tag="rcg")

        def emit_avs(o_ps, h, et, b):
            for i in range(2):
                kt = 2 * b + i
                for qt in range(NQT):
                    # start=True zeroes the whole 2KB bank; only the very
                    # first matmul of the head may set it
                    nc.tensor.matmul(
                        o_ps[:, qt, 0:25],
                        lhsT=et[:, i, qt * 128:(qt + 1) * 128],
                        rhs=vt_sb[:, kt, h, 0:25],
                        start=(kt == 0 and qt == 0), stop=(kt == PT - 1),
                        skip_group_check=True,
                    )

        def finish_head(o_ps, j, h):
            # denominators: column 24 of o_ps -> reciprocal -> one broadcast
            # multiply fuses division into the PSUM->SBUF move
            nc.vector.reciprocal(rc_g[:, :, j], o_ps[:, :, 24])
            nc.vector.tensor_tensor(
                out=o_sbT[:, :, h, :],
                in0=o_ps[:, :, 0:HD],
                in1=rc_g[:, :, j].unsqueeze(2).to_broadcast((128, NQT, HD)),
                op=AluOpType.mult,
            )

        deferred = []
        for j in range(4):
            h = 4 * g + j
            b0 = 32 * j
            defer = (g == 0 and j < 2)  # vt half0 still streaming during h0/h1
            if not defer:
                o_ps = ps_o.tile([128, NQT, 32], F32, tag="ops", name="o_ps")
            ets = []
            for b in range(4):  # kt pairs
                sps = ps_sps.tile([128, 2, QP], F32, tag="sps", name="sps")
                for i in range(2):
                    kt = 2 * b + i
                    ksrc = kgA if kt < 4 else kgB
                    nc.tensor.matmul(
                        sps[:, i, :],
                        lhsT=ksrc[b0:b0 + HD, (kt % 4) * 128:(kt % 4 + 1) * 128],
                        rhs=qg_sb[b0:b0 + HD, :],
                        start=True, stop=True, tile_position=(b0, 0),
                    )
                et = expp.tile([128, 2, QP], FP16, tag="exp", name="et")
                nc.scalar.activation(et[:, :, :], sps[:, :, :], Exp, scale=SCALE)
                # one vt tile per scores-slot in g0 (h0/h1); half1 paced at
                # every 4th slot across g1-g3 (g1 alone would starve the ACT)
                slot_n[0] += 1
                if g == 0 and j == 0 and b == 1:
                    gen_kB()
                if pending_vt and (defer or (g in (1, 2, 3) and slot_n[0] % 4 == 1)):
                    emit_vt_tile(*pending_vt.pop(0))
                # tail pre-work (transposes + partial proj) rides g7's slack
                if g == NG - 1 and pending_tail:
                    fn, args = pending_tail.pop(0)
                    fn(*args)
                if defer:
                    ets.append(et)
                else:
                    emit_avs(o_ps, h, et, b)
            if defer:
                deferred.append((j, h, ets))
            else:
                finish_head(o_ps, j, h)
            if g == 0 and j == 1:
                # vt half0 complete: run h0's and h1's avs now
                for dj, dh, dets in deferred:
                    o_ps = ps_o.tile([128, NQT, 32], F32, tag="ops", name="o_ps")
                    for b in range(4):
                        emit_avs(o_ps, dh, dets[b], b)
                    finish_head(o_ps, dj, dh)
                deferred = []

    if os.environ.get("KDBG", "0") == "1":
        nc.sync.dma_start(out=outs["dbg_osbt"], in_=o_sbT)
        nc.sync.dma_start(out=outs["dbg_vt"], in_=vt_sb[:, :, :, 0:25])
        nc.sync.dma_start(out=outs["dbg_rc"], in_=rc_g)

    # ---------------- tail: only the last channel chunk (ct5) remains ------
    for qt in range(NQT):
        emit_tp(qt, [5])
    outv = outs["out"].rearrange("(t p) (a b) -> t p a b", p=128, a=2)
    for qt in range(NQT):
        out_t = outp.tile([128, 2, 384], FP16, tag="out")
        for fh in range(2):
            # alternate psum pools so the proj matmuls don't wait on the adds
            pool, tg = (ps_gen, "gen") if (2 * qt + fh) % 2 == 0 else (ps_o, "ops")
            pp2 = pool.tile([128, 512], F32, tag=tg, name="pp2")
            nc.tensor.matmul(
                pp2[:, 0:384],
                lhsT=o_c[:, 5, qt, :],
                rhs=wp_sb[:, 5, fh * 384:(fh + 1) * 384],
                start=True, stop=True,
            )
            nc.vector.tensor_tensor(
                out=out_t[:, fh, :], in0=pp2[:, 0:384],
                in1=partialb[:, qt, fh, :], op=AluOpType.add)
        # alternate DMA queues so the 4 output copies overlap
        eng = nc.sync if qt % 2 == 0 else nc.gpsimd
        eng.dma_start(out=outv[qt], in_=out_t)

    ctx.close()


# ------------------------- host side -------------------------

def build_inmaps(x, w_qkv, b_qkv, w_proj, b_proj):
    x = np.ascontiguousarray(x, dtype=np.float32)
    w_qkv = np.asarray(w_qkv, dtype=np.float32)
    b_qkv = np.asarray(b_qkv, dtype=np.float32)
    w_proj = np.asarray(w_proj, dtype=np.float32)
    b_proj = np.asarray(b_proj, dtype=np.float32)

    w_q, w_k, w_v = w_qkv[:, :C], w_qkv[:, C:2 * C], w_qkv[:, 2 * C:]
    b_q, b_k, b_v = b_qkv[:C], b_qkv[C:2 * C], b_qkv[2 * C:]

    def pad_w(w):  # [768, 768] -> [768, 1024] with 24->32 head col padding
        out = np.zeros((C, NH, 32), dtype=np.float32)
        out[:, :, :HD] = w.reshape(C, NH, HD)
        return out.reshape(C, NH * 32)

    def pad_b(b):  # [768] -> [128, 8]
        out = np.zeros((4, 32, NG), dtype=np.float32)
        out[:, :HD, :] = b.reshape(NG, 4, HD).transpose(1, 2, 0)
        return out.reshape(128, NG)

    wk_g = pad_w(w_k).reshape(C, NG, 128).transpose(1, 0, 2)   # [NG, C, 128]
    wq_g = pad_w(w_q).reshape(C, NG, 128).transpose(1, 0, 2)
    wkq = np.concatenate([wk_g, wq_g], axis=2)                 # [NG, C, 256]
    # preswizzle to [NG, 128, CT, 256] so each partition's DMA read is contiguous
    wkq = np.ascontiguousarray(
        wkq.reshape(NG, CT, 128, 256).transpose(0, 2, 1, 3)).astype(np.float16)
    bk = pad_b(b_k)
    bq = pad_b(b_q)
    # b_v folded into the proj bias (attention weights sum to 1)
    bp1 = (b_proj + w_proj.T @ b_v).astype(np.float32)
    ident = np.eye(128, dtype=np.float16)

    in_maps = []
    for core in range(8):
        b, half = core // 2, core % 2
        xb = x[b].reshape(C, HW)
        # rotate so this core's queries are always columns 0:QP (keys are
        # permutation-invariant under softmax)
        xb = np.ascontiguousarray(np.roll(xb, -half * QP, axis=1)).astype(np.float16)
        in_maps.append({
            "x": xb,
            "wkq": wkq,
            "wv": np.ascontiguousarray(w_v).astype(np.float16),
            "wp": np.ascontiguousarray(w_proj).astype(np.float16),
            "bk": bk, "bq": bq, "bp1": bp1,
            "ident": ident,
        })
    return in_maps


_PROGRAM = None


def build_program():
    global _PROGRAM
    if _PROGRAM is not None:
        return _PROGRAM
    nc = bacc.Bacc("TRN2", target_bir_lowering=False, debug=False)
    ins = {
        "x": nc.dram_tensor("x", [C, HW], FP16, kind="ExternalInput").ap(),
        "wkq": nc.dram_tensor("wkq", [NG, 128, CT, 256], FP16, kind="ExternalInput").ap(),
        "wv": nc.dram_tensor("wv", [C, C], FP16, kind="ExternalInput").ap(),
        "wp": nc.dram_tensor("wp", [C, C], FP16, kind="ExternalInput").ap(),
        "bk": nc.dram_tensor("bk", [128, NG], F32, kind="ExternalInput").ap(),
        "bq": nc.dram_tensor("bq", [128, NG], F32, kind="ExternalInput").ap(),
        "bp1": nc.dram_tensor("bp1", [C], F32, kind="ExternalInput").ap(),
        "ident": nc.dram_tensor("ident", [128, 128], FP16, kind="ExternalInput").ap(),
    }
    outs = {"out": nc.dram_tensor("out", [QP, C], FP16, kind="ExternalOutput").ap()}
    if os.environ.get("KDBG", "0") == "1":
        outs["dbg_osbt"] = nc.dram_tensor(
            "dbg_osbt", [128, NQT, NH, HD], FP16, kind="ExternalOutput").ap()
        outs["dbg_vt"] = nc.dram_tensor(
            "dbg_vt", [128, PT, NH, 25], FP16, kind="ExternalOutput").ap()
        outs["dbg_rc"] = nc.dram_tensor(
            "dbg_rc", [128, NQT, 4], F32, kind="ExternalOutput").ap()
    with tile.TileContext(nc) as tc:
        emit_kernel(tc, outs, ins)
    nc.compile()
    _PROGRAM = nc
    return nc


def run(inputs, trace=False):
    nc = build_program()
    in_maps = build_inmaps(**inputs)
    try:
        res = bass_utils.run_bass_kernel_spmd(
            nc, in_maps, core_ids=list(range(8)), trace=trace)
    except ModuleNotFoundError:
        # BASS_TRACE path needs antenv.axon_hooks, absent in some containers;
        # rerun untraced rather than failing.
        prev = os.environ.get("BASS_NEVER_TRACE")
        os.environ["BASS_NEVER_TRACE"] = "1"
        try:
            res = bass_utils.run_bass_kernel_spmd(
                nc, in_maps, core_ids=list(range(8)), trace=False)
        finally:
            if prev is None:
                os.environ.pop("BASS_NEVER_TRACE", None)
            else:
                os.environ["BASS_NEVER_TRACE"] = prev
    out_full = np.empty((4, C, HW), dtype=np.float32)
    for core in range(8):
        b, half = core // 2, core % 2
        out_full[b][:, half * QP:(half + 1) * QP] = \
            res.results[core]["out"].astype(np.float32).T
    return out_full.reshape(4, C, 32, 32), res


def kernel(**inputs):
    out, _ = run(inputs, trace=False)
    return out


# revision 46
# speedup vs baseline: 1.0320x; 1.0088x over previous
"""Attention2d SPMD kernel for 8 TRN2 NeuronCores.

Problem (hardcoded): x [4, 768, 32, 32], w_qkv [768, 2304], b_qkv [2304],
w_proj [768, 768], b_proj [768]; 32 heads, head_dim 24.

Sharding: 8 cores = 4 batches x 2 query-halves (512 queries each).
Each core computes k/v for all 1024 positions of its batch (2x duplicated
across the pair of cores sharing a batch) and q/attention/proj for its own
512 query positions. Outputs are disjoint slices -> host gather is pure
concatenation (no collectives). Per-core x is ROTATED on the host so each
core's queries are always columns 0:512 (softmax is permutation-invariant
over keys), which makes the SPMD program identical across cores.

Per-core dataflow (per head-group g of 4 heads):
  k_g = w_k^T x  [128ch_pad, 1024]  (fp16)     q_g = w_q^T x  [128, 512]
  vT  = x^T w_v  [1024pos, 32 heads x (24ch | ones-col | 7 pad)]  (fp16)
  per head h, kt in 8 key-tiles: sT = k_h^T q_h [128k, 512q] -> Exp ->
    oT[128q-tile, 25] += et[:, qt]^T vT_h    (TRANSPOSED attn@v: queries on
    PSUM partitions, head_dim on the free axis -> 25-cycle matmuls; the
    vT ones-column lands the softmax denominator in oT column 24)
  divide: oT[:, 0:24] * (1/denom col) via one broadcast tensor_tensor per
    head (denominator is a per-partition column now - no DRAM bounce)
  tail: PE-transpose oT -> o [c, q] (identity matmul), then
    out^T[q, 768] = o^T W_p + b_p'   with b_p' = b_proj + W_p^T b_v folded
    on the host (exact: attention weights sum to 1). Host transposes out^T.

Precision: fp16 operands everywhere on the PE (1 cyc/row), fp32 PSUM,
denominator division exact fp32.
"""

import os
import numpy as np

import concourse.bacc as bacc
import concourse.bass as bass
import concourse.mybir as mybir
import concourse.tile as tile
from concourse import bass_utils
from concourse.alu_op_type import AluOpType

C = 768
HW = 1024
QP = 512          # queries per core
NH = 32           # heads
HD = 24           # head dim
NG = 8            # head groups (4 heads each, 32-padded rows)
CT = C // 128     # 6 contraction tiles
PT = HW // 128    # 8 position tiles
NQT = QP // 128   # 4 query tiles
SCALE = HD ** -0.5
BF16 = mybir.dt.bfloat16
FP16 = mybir.dt.float16
F32 = mybir.dt.float32


def emit_kernel(tc, outs, ins):
    from contextlib import ExitStack
    nc = tc.nc
    ctx = ExitStack()
    Exp = mybir.ActivationFunctionType.Exp

    big = ctx.enter_context(tc.tile_pool(name="big", bufs=1))
    kqp = ctx.enter_context(tc.tile_pool(name="kqp", bufs=2))
    wgp = ctx.enter_context(tc.tile_pool(name="wgp", bufs=3))
    expp = ctx.enter_context(tc.tile_pool(name="expp", bufs=8))
    smal = ctx.enter_context(tc.tile_pool(name="smal", bufs=2))
    outp = ctx.enter_context(tc.tile_pool(name="outp", bufs=4))
    # PSUM budget (8 banks): sps 2x[128,2,512]=4, gen 2x[128,512]=2,
    # oT 2x[128,4,32]=2.  Tail transpose/proj tiles reuse the sps slots.
    ps_sps = ctx.enter_context(tc.tile_pool(name="ps_sps", bufs=2, space="PSUM"))
    ps_gen = ctx.enter_context(tc.tile_pool(name="ps_gen", bufs=2, space="PSUM"))
    ps_o = ctx.enter_context(tc.tile_pool(name="ps_o", bufs=2, space="PSUM"))

    # ---------------- persistent SBUF tensors ----------------
    x_sb = big.tile([128, CT, HW], FP16)
    wv_sb = big.tile([128, CT, C], FP16)
    wp_sb = big.tile([128, CT, C], FP16)           # w_proj [c,f], c-chunked
    vt_sb = big.tile([128, PT, NH, 32], FP16)      # 2 MB; col HD is ones
    o_sbT = big.tile([128, NQT, NH, HD], FP16)     # divided o^T
    o_c = big.tile([128, CT, NQT, 128], FP16)      # transposed o (c on part)
    bk_sb = big.tile([128, NG], F32)
    bq_sb = big.tile([128, NG], F32)
    bp_bc = big.tile([128, C], F32)                # b_proj' bcast to all part
    ident = big.tile([128, 128], FP16)

    # DMA queues: SP carries ident + x (2 column-half DMAs: q/kA only need
    # cols 0:512, so the PE can start ~2.4us earlier) + the small tensors;
    # Pool carries the weight streams.  One DMA per tensor: each dma_start
    # pays ~1us of SWDGE fixed cost, so per-chunk DMAs serialize the start.
    xv = ins["x"].rearrange("(t p) n -> p t n", p=128)
    wvv = ins["wv"].rearrange("(t p) m -> p t m", p=128)
    warm_sb = big.tile([1, 2], F32)
    nc.vector.memset(warm_sb, 0.0)
    nc.scalar.activation(warm_sb[:, 1:2], warm_sb[:, 0:1], Exp, scale=1.0)
    nc.sync.dma_start(out=ident, in_=ins["ident"])
    nc.sync.dma_start(out=x_sb[:, :, 0:512], in_=xv[:, :, 0:512])
    nc.sync.dma_start(out=x_sb[:, :, 512:1024], in_=xv[:, :, 512:1024])
    nc.scalar.dma_start(out=bk_sb, in_=ins["bk"])
    nc.scalar.dma_start(out=bq_sb, in_=ins["bq"])
    nc.scalar.dma_start(out=bp_bc, in_=ins["bp1"].unsqueeze(0).to_broadcast((128, C)))
    # only vt column 24 (the denominator ones-column) is ever read beyond 0:24
    nc.vector.memset(vt_sb[:, :, :, 24:25], 1.0)
    # keep the PE continuously busy from ~t=2.5us so its p-state ramp
    # completes before the first real matmul
    warm_ps = ps_o.tile([128, 128], F32, tag="ops", name="warm_ps")
    for _ in range(30):
        nc.tensor.matmul(warm_ps, lhsT=ident, rhs=ident,
                         start=True, stop=True, skip_group_check=True)

    def emit_vt_tile(t, pt):
        # vT for heads 16t..16t+16 (dense, N=384) at position tile pt
        vps = ps_gen.tile([128, 512], F32, tag="gen", name="vps")
        for ct in range(CT):
            nc.tensor.matmul(
                vps[:, 0:384],
                lhsT=x_sb[:, ct, pt * 128:(pt + 1) * 128],
                rhs=wv_sb[:, ct, 384 * t:384 * (t + 1)],
                start=(ct == 0), stop=(ct == CT - 1),
            )
        nc.vector.tensor_copy(
            out=vt_sb[:, pt, 16 * t:16 * (t + 1), 0:HD],
            in_=vps[:, 0:384].rearrange("p (h d) -> p h d", d=HD),
        )

    # vT tiles pending emission: one per scores-slot during g0/g1 so the
    # PE never bursts 2+ vt tiles between exps (which would starve the ACT)
    pending_vt = [(0, pt) for pt in range(PT)] + [(1, pt) for pt in range(PT)]

    o_flat = o_sbT.rearrange("p a h d -> p a (h d)")
    partialb = big.tile([128, NQT, 2, 384], F32)   # proj(ct0..4) + bias

    def emit_tp(qt, cts):
        # PE-transpose o^T chunks -> o_c (c on partitions)
        nct = len(cts)
        tp = ps_gen.tile([128, nct, 128], FP16, tag="gen", name="tp")
        for k, ct in enumerate(cts):
            nc.tensor.matmul(
                tp[:, k, :],
                lhsT=o_flat[:, qt, ct * 128:(ct + 1) * 128],
                rhs=ident,
                is_transpose=True, start=(k == 0), stop=True,
                skip_group_check=True,
            )
        nc.vector.tensor_copy(out=o_c[:, cts[0]:cts[0] + nct, qt, :], in_=tp)

    def emit_pp1(qt, fh):
        # partial out^T = o^T(ct0..4) @ w_p half + bias, parked in SBUF
        pp1 = ps_gen.tile([128, 512], F32, tag="gen", name="pp1")
        for ct in range(CT - 1):
            nc.tensor.matmul(
                pp1[:, 0:384],
                lhsT=o_c[:, ct, qt, :],
                rhs=wp_sb[:, ct, fh * 384:(fh + 1) * 384],
                start=(ct == 0), stop=(ct == CT - 2),
            )
        nc.vector.tensor_tensor(
            out=partialb[:, qt, fh, :], in0=pp1[:, 0:384],
            in1=bp_bc[:, fh * 384:(fh + 1) * 384], op=AluOpType.add)

    pending_tail = [(emit_tp, (qt, [0, 1, 2, 3, 4])) for qt in range(NQT)] + \
                   [(emit_pp1, (qt, fh)) for qt in range(NQT) for fh in range(2)]
    slot_n = [0]

    # ---------------- per head-group: kq proj + attention ----------
    wkq0 = wgp.tile([128, CT, 256], FP16, tag="wkq", name="wkq0")
    nc.gpsimd.dma_start(out=wkq0, in_=ins["wkq"][0])
    # wv in column halves: vT half0 (heads 0-15) only needs cols 0:384, so
    # its first tiles aren't gated behind the full 9KB/partition transfer
    nc.gpsimd.dma_start(out=wv_sb[:, :, 0:384], in_=wvv[:, :, 0:384])
    nc.gpsimd.dma_start(out=wv_sb[:, :, 384:768], in_=wvv[:, :, 384:768])
    wkq_next = [None]
    for g in range(NG):
        if g == 0:
            wkq = wkq0
        else:
            wkq = wkq_next[0]
        wkg = wkq[:, :, 0:128]
        wqg = wkq[:, :, 128:256]

        qg_sb = kqp.tile([128, QP], FP16, tag="qg")
        kgA = kqp.tile([128, QP], FP16, tag="kgA")
        kgB = kqp.tile([128, QP], FP16, tag="kgB")
        qps = ps_gen.tile([128, 512], F32, tag="gen", name="qps")
        kpsA = ps_gen.tile([128, 512], F32, tag="gen", name="kpsA")
        # q and kA interleaved per x-quarter so g0 overlaps the x DMA chunks
        for cts in ((0, 3), (3, 6)):
            for ct in range(*cts):
                nc.tensor.matmul(
                    qps[:, :], lhsT=wqg[:, ct, :], rhs=x_sb[:, ct, 0:QP],
                    start=(ct == 0), stop=(ct == CT - 1),
                )
            for ct in range(*cts):
                nc.tensor.matmul(
                    kpsA[:, :], lhsT=wkg[:, ct, :], rhs=x_sb[:, ct, 0:QP],
                    start=(ct == 0), stop=(ct == CT - 1),
                )
        nc.vector.tensor_scalar_add(qg_sb[:, :], qps, bq_sb[:, g:g + 1])
        if g == 0:
            # ACT is idle pre-softmax and Identity shares Exp's act table:
            # overlap the kgA move with the qg move instead of serializing DVE
            Ident = mybir.ActivationFunctionType.Identity
            nc.scalar.activation(kgA[:, :], kpsA, Ident,
                                 bias=bk_sb[:, g:g + 1], scale=1.0)
        else:
            nc.vector.tensor_scalar_add(kgA[:, :], kpsA, bk_sb[:, g:g + 1])

        def gen_kB():
            kpsB = ps_gen.tile([128, 512], F32, tag="gen", name="kpsB")
            for ct in range(CT):
                nc.tensor.matmul(
                    kpsB[:, :], lhsT=wkg[:, ct, :], rhs=x_sb[:, ct, 512:1024],
                    start=(ct == 0), stop=(ct == CT - 1),
                )
            nc.vector.tensor_scalar_add(kgB[:, :], kpsB, bk_sb[:, g:g + 1])

        if g > 0:
            gen_kB()
        # for g0, kB waits on the second x half-DMA; deferring it into h0's
        # b1 slot keeps it from gating the first scores/exps

        if g == 2:
            wpv = ins["wp"].rearrange("(t p) m -> p t m", p=128)
            nc.gpsimd.dma_start(out=wp_sb, in_=wpv)

        rc_g = smal.tile([128, NQT, 4], F32, tag="rcg")

        def emit_avs(o_ps, h, et, b):
            for i in range(2):
                kt = 2 * b + i
                for qt in range(NQT):
                    # start=True zeroes the whole 2KB bank; only the very
                    # first matmul of the head may set it
                    nc.tensor.matmul(
                        o_ps[:, qt, 0:25],
                        lhsT=et[:, i, qt * 128:(qt + 1) * 128],
                        rhs=vt_sb[:, kt, h, 0:25],
                        start=(kt == 0 and qt == 0), stop=(kt == PT - 1),
                        skip_group_check=True,
                    )

        def finish_head(o_ps, j, h):
            # denominators: column 24 of o_ps -> reciprocal -> one broadcast
            # multiply fuses division into the PSUM->SBUF move
            nc.vector.reciprocal(rc_g[:, :, j], o_ps[:, :, 24])
            nc.vector.tensor_tensor(
                out=o_sbT[:, :, h, :],
                in0=o_ps[:, :, 0:HD],
                in1=rc_g[:, :, j].unsqueeze(2).to_broadcast((128, NQT, HD)),
                op=AluOpType.mult,
            )

        deferred = []
        for j in range(4):
            h = 4 * g + j
            b0 = 32 * j
            if j == 2 and g < NG - 1:
                # prefetch next group's weights mid-group: at group-top the
                # serialized DMA stream would bump xh1/wv behind them
                wkq_next[0] = wgp.tile([128, CT, 256], FP16, tag="wkq",
                                       name="wkq_pre")
                nc.gpsimd.dma_start(out=wkq_next[0], in_=ins["wkq"][g + 1])
            defer = (g == 0 and j < 2)  # vt half0 still streaming during h0/h1
            if not defer:
                o_ps = ps_o.tile([128, NQT, 32], F32, tag="ops", name="o_ps")
            ets = []
            for b in range(4):  # kt pairs
                sps = ps_sps.tile([128, 2, QP], F32, tag="sps", name="sps")
                for i in range(2):
                    kt = 2 * b + i
                    ksrc = kgA if kt < 4 else kgB
                    nc.tensor.matmul(
                        sps[:, i, :],
                        lhsT=ksrc[b0:b0 + HD, (kt % 4) * 128:(kt % 4 + 1) * 128],
                        rhs=qg_sb[b0:b0 + HD, :],
                        start=True, stop=True, tile_position=(b0, 0),
                    )
                et = expp.tile([128, 2, QP], FP16, tag="exp", name="et")
                nc.scalar.activation(et[:, :, :], sps[:, :, :], Exp, scale=SCALE)
                # one vt tile per scores-slot in g0 (h0/h1); half1 paced at
                # every 4th slot across g1-g3 (g1 alone would starve the ACT)
                slot_n[0] += 1
                if g == 0 and j == 0 and b == 1:
                    gen_kB()
                if pending_vt and (defer or (g in (1, 2, 3) and slot_n[0] % 4 == 1)):
                    emit_vt_tile(*pending_vt.pop(0))
                # tail pre-work (transposes + partial proj) rides g7's slack
                if g == NG - 1 and pending_tail:
                    fn, args = pending_tail.pop(0)
                    fn(*args)
                if defer:
                    ets.append(et)
                else:
                    emit_avs(o_ps, h, et, b)
            if defer:
                deferred.append((j, h, ets))
            else:
                finish_head(o_ps, j, h)
            if g == 0 and j == 1:
                # vt half0 complete: run h0's and h1's avs now
                for dj, dh, dets in deferred:
                    o_ps = ps_o.tile([128, NQT, 32], F32, tag="ops", name="o_ps")
                    for b in range(4):
                        emit_avs(o_ps, dh, dets[b], b)
                    finish_head(o_ps, dj, dh)
                deferred = []

    if os.environ.get("KDBG", "0") == "1":
        nc.sync.dma_start(out=outs["dbg_osbt"], in_=o_sbT)
        nc.sync.dma_start(out=outs["dbg_vt"], in_=vt_sb[:, :, :, 0:25])
        nc.sync.dma_start(out=outs["dbg_rc"], in_=rc_g)

    # ---------------- tail: only the last channel chunk (ct5) remains ------
    for qt in range(NQT):
        emit_tp(qt, [5])
    outv = outs["out"].rearrange("(t p) (a b) -> t p a b", p=128, a=2)
    for qt in range(NQT):
        out_t = outp.tile([128, 2, 384], FP16, tag="out")
        for fh in range(2):
            # alternate psum pools so the proj matmuls don't wait on the adds
            pool, tg = (ps_gen, "gen") if (2 * qt + fh) % 2 == 0 else (ps_o, "ops")
            pp2 = pool.tile([128, 512], F32, tag=tg, name="pp2")
            nc.tensor.matmul(
                pp2[:, 0:384],
                lhsT=o_c[:, 5, qt, :],
                rhs=wp_sb[:, 5, fh * 384:(fh + 1) * 384],
                start=True, stop=True,
            )
            nc.vector.tensor_tensor(
                out=out_t[:, fh, :], in0=pp2[:, 0:384],
                in1=partialb[:, qt, fh, :], op=AluOpType.add)
        # alternate DMA queues so the 4 output copies overlap
        eng = nc.sync if qt % 2 == 0 else nc.gpsimd
        eng.dma_start(out=outv[qt], in_=out_t)

    ctx.close()


# ------------------------- host side -------------------------

def build_inmaps(x, w_qkv, b_qkv, w_proj, b_proj):
    x = np.ascontiguousarray(x, dtype=np.float32)
    w_qkv = np.asarray(w_qkv, dtype=np.float32)
    b_qkv = np.asarray(b_qkv, dtype=np.float32)
    w_proj = np.asarray(w_proj, dtype=np.float32)
    b_proj = np.asarray(b_proj, dtype=np.float32)

    w_q, w_k, w_v = w_qkv[:, :C], w_qkv[:, C:2 * C], w_qkv[:, 2 * C:]
    b_q, b_k, b_v = b_qkv[:C], b_qkv[C:2 * C], b_qkv[2 * C:]

    def pad_w(w):  # [768, 768] -> [768, 1024] with 24->32 head col padding
        out = np.zeros((C, NH, 32), dtype=np.float32)
        out[:, :, :HD] = w.reshape(C, NH, HD)
        return out.reshape(C, NH * 32)

    def pad_b(b):  # [768] -> [128, 8]
        out = np.zeros((4, 32, NG), dtype=np.float32)
        out[:, :HD, :] = b.reshape(NG, 4, HD).transpose(1, 2, 0)
        return out.reshape(128, NG)

    wk_g = pad_w(w_k).reshape(C, NG, 128).transpose(1, 0, 2)   # [NG, C, 128]
    wq_g = pad_w(w_q).reshape(C, NG, 128).transpose(1, 0, 2)
    wkq = np.concatenate([wk_g, wq_g], axis=2)                 # [NG, C, 256]
    # preswizzle to [NG, 128, CT, 256] so each partition's DMA read is contiguous
    wkq = np.ascontiguousarray(
        wkq.reshape(NG, CT, 128, 256).transpose(0, 2, 1, 3)).astype(np.float16)
    bk = pad_b(b_k)
    bq = pad_b(b_q)
    # b_v folded into the proj bias (attention weights sum to 1)
    bp1 = (b_proj + w_proj.T @ b_v).astype(np.float32)
    ident = np.eye(128, dtype=np.float16)

    in_maps = []
    for core in range(8):
        b, half = core // 2, core % 2
        xb = x[b].reshape(C, HW)
        # rotate so this core's queries are always columns 0:QP (keys are
        # permutation-invariant under softmax)
        xb = np.ascontiguousarray(np.roll(xb, -half * QP, axis=1)).astype(np.float16)
        in_maps.append({
            "x": xb,
            "wkq": wkq,
            "wv": np.ascontiguousarray(w_v).astype(np.float16),
            "wp": np.ascontiguousarray(w_proj).astype(np.float16),
            "bk": bk, "bq": bq, "bp1": bp1,
            "ident": ident,
        })
    return in_maps


_PROGRAM = None


def build_program():
    global _PROGRAM
    if _PROGRAM is not None:
        return _PROGRAM
    nc = bacc.Bacc("TRN2", target_bir_lowering=False, debug=False)
    ins = {
        "x": nc.dram_tensor("x", [C, HW], FP16, kind="ExternalInput").ap(),
        "wkq": nc.dram_tensor("wkq", [NG, 128, CT, 256], FP16, kind="ExternalInput").ap(),
        "wv": nc.dram_tensor("wv", [C, C], FP16, kind="ExternalInput").ap(),
        "wp": nc.dram_tensor("wp", [C, C], FP16, kind="ExternalInput").ap(),
        "bk": nc.dram_tensor("bk", [128, NG], F32, kind="ExternalInput").ap(),
        "bq": nc.dram_tensor("bq", [128, NG], F32, kind="ExternalInput").ap(),
        "bp1": nc.dram_tensor("bp1", [C], F32, kind="ExternalInput").ap(),
        "ident": nc.dram_tensor("ident", [128, 128], FP16, kind="ExternalInput").ap(),
    }
    outs = {"out": nc.dram_tensor("out", [QP, C], FP16, kind="ExternalOutput").ap()}
    if os.environ.get("KDBG", "0") == "1":
        outs["dbg_osbt"] = nc.dram_tensor(
            "dbg_osbt", [128, NQT, NH, HD], FP16, kind="ExternalOutput").ap()
        outs["dbg_vt"] = nc.dram_tensor(
            "dbg_vt", [128, PT, NH, 25], FP16, kind="ExternalOutput").ap()
        outs["dbg_rc"] = nc.dram_tensor(
            "dbg_rc", [128, NQT, 4], F32, kind="ExternalOutput").ap()
    with tile.TileContext(nc) as tc:
        emit_kernel(tc, outs, ins)
    nc.compile()
    _PROGRAM = nc
    return nc


def run(inputs, trace=False):
    nc = build_program()
    in_maps = build_inmaps(**inputs)
    try:
        res = bass_utils.run_bass_kernel_spmd(
            nc, in_maps, core_ids=list(range(8)), trace=trace)
    except ModuleNotFoundError:
        # BASS_TRACE path needs antenv.axon_hooks, absent in some containers;
        # rerun untraced rather than failing.
        prev = os.environ.get("BASS_NEVER_TRACE")
        os.environ["BASS_NEVER_TRACE"] = "1"
        try:
            res = bass_utils.run_bass_kernel_spmd(
                nc, in_maps, core_ids=list(range(8)), trace=False)
        finally:
            if prev is None:
                os.environ.pop("BASS_NEVER_TRACE", None)
            else:
                os.environ["BASS_NEVER_TRACE"] = prev
    out_full = np.empty((4, C, HW), dtype=np.float32)
    for core in range(8):
        b, half = core // 2, core % 2
        out_full[b][:, half * QP:(half + 1) * QP] = \
            res.results[core]["out"].astype(np.float32).T
    return out_full.reshape(4, C, 32, 32), res


def kernel(**inputs):
    out, _ = run(inputs, trace=False)
    return out


# revision 49
# speedup vs baseline: 1.0563x; 1.0236x over previous
"""Attention2d SPMD kernel for 8 TRN2 NeuronCores.

Problem (hardcoded): x [4, 768, 32, 32], w_qkv [768, 2304], b_qkv [2304],
w_proj [768, 768], b_proj [768]; 32 heads, head_dim 24.

Sharding: 8 cores = 4 batches x 2 query-halves (512 queries each).
Each core computes k/v for all 1024 positions of its batch (2x duplicated
across the pair of cores sharing a batch) and q/attention/proj for its own
512 query positions. Outputs are disjoint slices -> host gather is pure
concatenation (no collectives). Per-core x is ROTATED on the host so each
core's queries are always columns 0:512 (softmax is permutation-invariant
over keys), which makes the SPMD program identical across cores.

Per-core dataflow (per head-group g of 4 heads):
  k_g = w_k^T x  [128ch_pad, 1024]  (fp16)     q_g = w_q^T x  [128, 512]
  vT  = x^T w_v  [1024pos, 32 heads x (24ch | ones-col | 7 pad)]  (fp16)
  per head h, kt in 8 key-tiles: sT = k_h^T q_h [128k, 512q] -> Exp ->
    oT[128q-tile, 25] += et[:, qt]^T vT_h    (TRANSPOSED attn@v: queries on
    PSUM partitions, head_dim on the free axis -> 25-cycle matmuls; the
    vT ones-column lands the softmax denominator in oT column 24)
  divide: oT[:, 0:24] * (1/denom col) via one broadcast tensor_tensor per
    head (denominator is a per-partition column now - no DRAM bounce)
  tail: PE-transpose oT -> o [c, q] (identity matmul), then
    out^T[q, 768] = o^T W_p + b_p'   with b_p' = b_proj + W_p^T b_v folded
    on the host (exact: attention weights sum to 1). Host transposes out^T.

Precision: fp16 operands everywhere on the PE (1 cyc/row), fp32 PSUM,
denominator division exact fp32.
"""

import os
import numpy as np

import concourse.bacc as bacc
import concourse.bass as bass
import concourse.mybir as mybir
import concourse.tile as tile
from concourse import bass_utils
from concourse.alu_op_type import AluOpType

C = 768
HW = 1024
QP = 512          # queries per core
NH = 32           # heads
HD = 24           # head dim
NG = 8            # head groups (4 heads each, 32-padded rows)
CT = C // 128     # 6 contraction tiles
PT = HW // 128    # 8 position tiles
NQT = QP // 128   # 4 query tiles
SCALE = HD ** -0.5
BF16 = mybir.dt.bfloat16
FP16 = mybir.dt.float16
F32 = mybir.dt.float32


def emit_kernel(tc, outs, ins):
    from contextlib import ExitStack
    nc = tc.nc
    ctx = ExitStack()
    Exp = mybir.ActivationFunctionType.Exp

    big = ctx.enter_context(tc.tile_pool(name="big", bufs=1))
    kqp = ctx.enter_context(tc.tile_pool(name="kqp", bufs=2))
    wgp = ctx.enter_context(tc.tile_pool(name="wgp", bufs=3))
    expp = ctx.enter_context(tc.tile_pool(name="expp", bufs=8))
    smal = ctx.enter_context(tc.tile_pool(name="smal", bufs=2))
    outp = ctx.enter_context(tc.tile_pool(name="outp", bufs=4))
    # PSUM budget (8 banks): sps 2x[128,2,512]=4, gen 2x[128,512]=2,
    # oT 2x[128,4,32]=2.  Tail transpose/proj tiles reuse the sps slots.
    ps_sps = ctx.enter_context(tc.tile_pool(name="ps_sps", bufs=2, space="PSUM"))
    ps_gen = ctx.enter_context(tc.tile_pool(name="ps_gen", bufs=2, space="PSUM"))
    ps_o = ctx.enter_context(tc.tile_pool(name="ps_o", bufs=2, space="PSUM"))

    # ---------------- persistent SBUF tensors ----------------
    x_sb = big.tile([128, CT, HW], FP16)
    wv_sb = big.tile([128, CT, C], FP16)
    wp_sb = big.tile([128, CT, C], FP16)           # w_proj [c,f], c-chunked
    vt_sb = big.tile([128, PT, NH, 32], FP16)      # 2 MB; col HD is ones
    o_sbT = big.tile([128, NQT, NH, HD], FP16)     # divided o^T
    o_c = big.tile([128, CT, NQT, 128], FP16)      # transposed o (c on part)
    bk_sb = big.tile([128, NG], F32)
    bq_sb = big.tile([128, NG], F32)
    bp_bc = big.tile([128, C], F32)                # b_proj' bcast to all part
    ident = big.tile([128, 128], FP16)

    # DMA queues: SP carries ident + x (2 column-half DMAs: q/kA only need
    # cols 0:512, so the PE can start ~2.4us earlier) + the small tensors;
    # Pool carries the weight streams.  One DMA per tensor: each dma_start
    # pays ~1us of SWDGE fixed cost, so per-chunk DMAs serialize the start.
    xv = ins["x"].rearrange("(t p) n -> p t n", p=128)
    wvv = ins["wv"].rearrange("(t p) m -> p t m", p=128)
    warm_sb = big.tile([1, 2], F32)
    nc.vector.memset(warm_sb, 0.0)
    nc.scalar.activation(warm_sb[:, 1:2], warm_sb[:, 0:1], Exp, scale=1.0)
    nc.sync.dma_start(out=ident, in_=ins["ident"])
    nc.sync.dma_start(out=x_sb[:, :, 0:512], in_=xv[:, :, 0:512])
    nc.sync.dma_start(out=x_sb[:, :, 512:1024], in_=xv[:, :, 512:1024])
    nc.scalar.dma_start(out=bk_sb, in_=ins["bk"])
    nc.scalar.dma_start(out=bq_sb, in_=ins["bq"])
    nc.scalar.dma_start(out=bp_bc, in_=ins["bp1"].unsqueeze(0).to_broadcast((128, C)))
    # only vt column 24 (the denominator ones-column) is ever read beyond 0:24
    nc.vector.memset(vt_sb[:, :, :, 24:25], 1.0)
    # keep the PE continuously busy from ~t=2.5us so its p-state ramp
    # completes before the first real matmul
    warm_ps = ps_o.tile([128, 128], F32, tag="ops", name="warm_ps")
    for _ in range(30):
        nc.tensor.matmul(warm_ps, lhsT=ident, rhs=ident,
                         start=True, stop=True, skip_group_check=True)

    def emit_vt_strip(q, pt):
        # vT for the 4 heads of group q at position tile pt: 96-wide matmuls
        # (0.24us) fit any scores-slot without starving the ACT engine
        vps = ps_gen.tile([128, 512], F32, tag="gen", name="vps")
        for ct in range(CT):
            nc.tensor.matmul(
                vps[:, 0:96],
                lhsT=x_sb[:, ct, pt * 128:(pt + 1) * 128],
                rhs=wv_sb[:, ct, 96 * q:96 * (q + 1)],
                start=(ct == 0), stop=(ct == CT - 1),
            )
        nc.vector.tensor_copy(
            out=vt_sb[:, pt, 4 * q:4 * (q + 1), 0:HD],
            in_=vps[:, 0:96].rearrange("p (h d) -> p h d", d=HD),
        )

    # vT strips pending emission, group-quarter-major: group g's strips are
    # fully emitted before group g's avs need them
    pending_vt = [(q, pt) for q in range(NG) for pt in range(PT)]

    o_flat = o_sbT.rearrange("p a h d -> p a (h d)")
    partialb = big.tile([128, NQT, 2, 384], F32)   # proj(ct0..4) + bias

    def emit_tp(qt, cts):
        # PE-transpose o^T chunks -> o_c (c on partitions)
        nct = len(cts)
        tp = ps_gen.tile([128, nct, 128], FP16, tag="gen", name="tp")
        for k, ct in enumerate(cts):
            nc.tensor.matmul(
                tp[:, k, :],
                lhsT=o_flat[:, qt, ct * 128:(ct + 1) * 128],
                rhs=ident,
                is_transpose=True, start=(k == 0), stop=True,
                skip_group_check=True,
            )
        nc.vector.tensor_copy(out=o_c[:, cts[0]:cts[0] + nct, qt, :], in_=tp)

    def emit_pp1(qt, fh):
        # partial out^T = o^T(ct0..4) @ w_p half + bias, parked in SBUF
        pp1 = ps_gen.tile([128, 512], F32, tag="gen", name="pp1")
        for ct in range(CT - 1):
            nc.tensor.matmul(
                pp1[:, 0:384],
                lhsT=o_c[:, ct, qt, :],
                rhs=wp_sb[:, ct, fh * 384:(fh + 1) * 384],
                start=(ct == 0), stop=(ct == CT - 2),
            )
        nc.vector.tensor_tensor(
            out=partialb[:, qt, fh, :], in0=pp1[:, 0:384],
            in1=bp_bc[:, fh * 384:(fh + 1) * 384], op=AluOpType.add)

    pending_tail = [(emit_tp, (qt, [0, 1, 2, 3, 4])) for qt in range(NQT)] + \
                   [(emit_pp1, (qt, fh)) for qt in range(NQT) for fh in range(2)]
    slot_n = [0]

    # ---------------- per head-group: kq proj + attention ----------
    wkq0 = wgp.tile([128, CT, 256], FP16, tag="wkq", name="wkq0")
    nc.gpsimd.dma_start(out=wkq0, in_=ins["wkq"][0])
    # wv in column halves: vT half0 (heads 0-15) only needs cols 0:384, so
    # its first tiles aren't gated behind the full 9KB/partition transfer
    nc.gpsimd.dma_start(out=wv_sb[:, :, 0:384], in_=wvv[:, :, 0:384])
    nc.gpsimd.dma_start(out=wv_sb[:, :, 384:768], in_=wvv[:, :, 384:768])
    wkq_next = [None]
    for g in range(NG):
        if g == 0:
            wkq = wkq0
        else:
            wkq = wkq_next[0]
        wkg = wkq[:, :, 0:128]
        wqg = wkq[:, :, 128:256]

        qg_sb = kqp.tile([128, QP], FP16, tag="qg")
        kgA = kqp.tile([128, QP], FP16, tag="kgA")
        kgB = kqp.tile([128, QP], FP16, tag="kgB")
        qps = ps_gen.tile([128, 512], F32, tag="gen", name="qps")
        kpsA = ps_gen.tile([128, 512], F32, tag="gen", name="kpsA")
        # q and kA interleaved per x-quarter so g0 overlaps the x DMA chunks
        for cts in ((0, 3), (3, 6)):
            for ct in range(*cts):
                nc.tensor.matmul(
                    qps[:, :], lhsT=wqg[:, ct, :], rhs=x_sb[:, ct, 0:QP],
                    start=(ct == 0), stop=(ct == CT - 1),
                )
            for ct in range(*cts):
                nc.tensor.matmul(
                    kpsA[:, :], lhsT=wkg[:, ct, :], rhs=x_sb[:, ct, 0:QP],
                    start=(ct == 0), stop=(ct == CT - 1),
                )
        nc.vector.tensor_scalar_add(qg_sb[:, :], qps, bq_sb[:, g:g + 1])
        if g == 0:
            # ACT is idle pre-softmax and Identity shares Exp's act table:
            # overlap the kgA move with the qg move instead of serializing DVE
            Ident = mybir.ActivationFunctionType.Identity
            nc.scalar.activation(kgA[:, :], kpsA, Ident,
                                 bias=bk_sb[:, g:g + 1], scale=1.0)
        else:
            nc.vector.tensor_scalar_add(kgA[:, :], kpsA, bk_sb[:, g:g + 1])

        def gen_kB():
            kpsB = ps_gen.tile([128, 512], F32, tag="gen", name="kpsB")
            for ct in range(CT):
                nc.tensor.matmul(
                    kpsB[:, :], lhsT=wkg[:, ct, :], rhs=x_sb[:, ct, 512:1024],
                    start=(ct == 0), stop=(ct == CT - 1),
                )
            nc.vector.tensor_scalar_add(kgB[:, :], kpsB, bk_sb[:, g:g + 1])

        if g > 0:
            gen_kB()
        # for g0, kB waits on the second x half-DMA; deferring it into h0's
        # b1 slot keeps it from gating the first scores/exps

        if g == 2:
            wpv = ins["wp"].rearrange("(t p) m -> p t m", p=128)
            nc.gpsimd.dma_start(out=wp_sb, in_=wpv)

        rc_g = smal.tile([128, NQT, 4], F32, tag="rcg")

        def emit_avs(o_ps, h, et, b):
            for i in range(2):
                kt = 2 * b + i
                for qt in range(NQT):
                    # start=True zeroes the whole 2KB bank; only the very
                    # first matmul of the head may set it
                    nc.tensor.matmul(
                        o_ps[:, qt, 0:25],
                        lhsT=et[:, i, qt * 128:(qt + 1) * 128],
                        rhs=vt_sb[:, kt, h, 0:25],
                        start=(kt == 0 and qt == 0), stop=(kt == PT - 1),
                        skip_group_check=True,
                    )

        def finish_head(o_ps, j, h):
            # denominators: column 24 of o_ps -> reciprocal -> one broadcast
            # multiply fuses division into the PSUM->SBUF move
            nc.vector.reciprocal(rc_g[:, :, j], o_ps[:, :, 24])
            nc.vector.tensor_tensor(
                out=o_sbT[:, :, h, :],
                in0=o_ps[:, :, 0:HD],
                in1=rc_g[:, :, j].unsqueeze(2).to_broadcast((128, NQT, HD)),
                op=AluOpType.mult,
            )

        deferred = []
        for j in range(4):
            h = 4 * g + j
            b0 = 32 * j
            if j == 2 and g < NG - 1:
                # prefetch next group's weights mid-group: at group-top the
                # serialized DMA stream would bump xh1/wv behind them
                wkq_next[0] = wgp.tile([128, CT, 256], FP16, tag="wkq",
                                       name="wkq_pre")
                nc.gpsimd.dma_start(out=wkq_next[0], in_=ins["wkq"][g + 1])
            defer = (g == 0 and j < 2)  # vt half0 still streaming during h0/h1
            if not defer:
                o_ps = ps_o.tile([128, NQT, 32], F32, tag="ops", name="o_ps")
            ets = []
            for b in range(4):  # kt pairs
                sps = ps_sps.tile([128, 2, QP], F32, tag="sps", name="sps")
                for i in range(2):
                    kt = 2 * b + i
                    ksrc = kgA if kt < 4 else kgB
                    nc.tensor.matmul(
                        sps[:, i, :],
                        lhsT=ksrc[b0:b0 + HD, (kt % 4) * 128:(kt % 4 + 1) * 128],
                        rhs=qg_sb[b0:b0 + HD, :],
                        start=True, stop=True, tile_position=(b0, 0),
                    )
                et = expp.tile([128, 2, QP], FP16, tag="exp", name="et")
                nc.scalar.activation(et[:, :, :], sps[:, :, :], Exp, scale=SCALE)
                # one vt tile per scores-slot in g0 (h0/h1); half1 paced at
                # every 4th slot across g1-g3 (g1 alone would starve the ACT)
                slot_n[0] += 1
                if g == 0 and j == 0 and b == 1:
                    gen_kB()
                if pending_vt and (g == 0 or slot_n[0] % 2 == 1):
                    emit_vt_strip(*pending_vt.pop(0))
                # tail pre-work (transposes + partial proj) rides g7's slack
                if g == NG - 1 and pending_tail:
                    fn, args = pending_tail.pop(0)
                    fn(*args)
                if defer:
                    ets.append(et)
                else:
                    emit_avs(o_ps, h, et, b)
            if defer:
                deferred.append((j, h, ets))
            else:
                finish_head(o_ps, j, h)
            if g == 0 and j == 1:
                # vt half0 complete: run h0's and h1's avs now
                for dj, dh, dets in deferred:
                    o_ps = ps_o.tile([128, NQT, 32], F32, tag="ops", name="o_ps")
                    for b in range(4):
                        emit_avs(o_ps, dh, dets[b], b)
                    finish_head(o_ps, dj, dh)
                deferred = []

    if os.environ.get("KDBG", "0") == "1":
        nc.sync.dma_start(out=outs["dbg_osbt"], in_=o_sbT)
        nc.sync.dma_start(out=outs["dbg_vt"], in_=vt_sb[:, :, :, 0:25])
        nc.sync.dma_start(out=outs["dbg_rc"], in_=rc_g)

    # ---------------- tail: only the last channel chunk (ct5) remains ------
    for qt in range(NQT):
        emit_tp(qt, [5])
    outv = outs["out"].rearrange("(t p) (a b) -> t p a b", p=128, a=2)
    for qt in range(NQT):
        out_t = outp.tile([128, 2, 384], FP16, tag="out")
        # sps banks are free once the last exp has read them: 2-bank pp2
        # tiles let one fused add per qt replace two half adds
        pp2 = ps_sps.tile([128, 2, QP], F32, tag="sps", name="pp2")
        for fh in range(2):
            nc.tensor.matmul(
                pp2[:, fh, 0:384],
                lhsT=o_c[:, 5, qt, :],
                rhs=wp_sb[:, 5, fh * 384:(fh + 1) * 384],
                start=True, stop=True,
            )
        nc.vector.tensor_tensor(
            out=out_t, in0=pp2[:, :, 0:384],
            in1=partialb[:, qt, :, :], op=AluOpType.add)
        # HWDGE queues (SP/ACT) generate descriptors ~0.4us faster than
        # the Pool SWDGE path
        eng = nc.sync if qt % 2 == 0 else nc.scalar
        eng.dma_start(out=outv[qt], in_=out_t)

    ctx.close()


# ------------------------- host side -------------------------

def build_inmaps(x, w_qkv, b_qkv, w_proj, b_proj):
    x = np.ascontiguousarray(x, dtype=np.float32)
    w_qkv = np.asarray(w_qkv, dtype=np.float32)
    b_qkv = np.asarray(b_qkv, dtype=np.float32)
    w_proj = np.asarray(w_proj, dtype=np.float32)
    b_proj = np.asarray(b_proj, dtype=np.float32)

    w_q, w_k, w_v = w_qkv[:, :C], w_qkv[:, C:2 * C], w_qkv[:, 2 * C:]
    b_q, b_k, b_v = b_qkv[:C], b_qkv[C:2 * C], b_qkv[2 * C:]

    def pad_w(w):  # [768, 768] -> [768, 1024] with 24->32 head col padding
        out = np.zeros((C, NH, 32), dtype=np.float32)
        out[:, :, :HD] = w.reshape(C, NH, HD)
        return out.reshape(C, NH * 32)

    def pad_b(b):  # [768] -> [128, 8]
        out = np.zeros((4, 32, NG), dtype=np.float32)
        out[:, :HD, :] = b.reshape(NG, 4, HD).transpose(1, 2, 0)
        return out.reshape(128, NG)

    wk_g = pad_w(w_k).reshape(C, NG, 128).transpose(1, 0, 2)   # [NG, C, 128]
    wq_g = pad_w(w_q).reshape(C, NG, 128).transpose(1, 0, 2)
    wkq = np.concatenate([wk_g, wq_g], axis=2)                 # [NG, C, 256]
    # preswizzle to [NG, 128, CT, 256] so each partition's DMA read is contiguous
    wkq = np.ascontiguousarray(
        wkq.reshape(NG, CT, 128, 256).transpose(0, 2, 1, 3)).astype(np.float16)
    bk = pad_b(b_k)
    bq = pad_b(b_q)
    # b_v folded into the proj bias (attention weights sum to 1)
    bp1 = (b_proj + w_proj.T @ b_v).astype(np.float32)
    ident = np.eye(128, dtype=np.float16)

    in_maps = []
    for core in range(8):
        b, half = core // 2, core % 2
        xb = x[b].reshape(C, HW)
        # rotate so this core's queries are always columns 0:QP (keys are
        # permutation-invariant under softmax)
        xb = np.ascontiguousarray(np.roll(xb, -half * QP, axis=1)).astype(np.float16)
        in_maps.append({
            "x": xb,
            "wkq": wkq,
            "wv": np.ascontiguousarray(w_v).astype(np.float16),
            "wp": np.ascontiguousarray(w_proj).astype(np.float16),
            "bk": bk, "bq": bq, "bp1": bp1,
            "ident": ident,
        })
    return in_maps


_PROGRAM = None


def build_program():
    global _PROGRAM
    if _PROGRAM is not None:
        return _PROGRAM
    nc = bacc.Bacc("TRN2", target_bir_lowering=False, debug=False)
    ins = {
        "x": nc.dram_tensor("x", [C, HW], FP16, kind="ExternalInput").ap(),
        "wkq": nc.dram_tensor("wkq", [NG, 128, CT, 256], FP16, kind="ExternalInput").ap(),
        "wv": nc.dram_tensor("wv", [C, C], FP16, kind="ExternalInput").ap(),
        "wp": nc.dram_tensor("wp", [C, C], FP16, kind="ExternalInput").ap(),
        "bk": nc.dram_tensor("bk", [128, NG], F32, kind="ExternalInput").ap(),
        "bq": nc.dram_tensor("bq", [128, NG], F32, kind="ExternalInput").ap(),
        "bp1": nc.dram_tensor("bp1", [C], F32, kind="ExternalInput").ap(),
        "ident": nc.dram_tensor("ident", [128, 128], FP16, kind="ExternalInput").ap(),
    }
    outs = {"out": nc.dram_tensor("out", [QP, C], FP16, kind="ExternalOutput").ap()}
    if os.environ.get("KDBG", "0") == "1":
        outs["dbg_osbt"] = nc.dram_tensor(
            "dbg_osbt", [128, NQT, NH, HD], FP16, kind="ExternalOutput").ap()
        outs["dbg_vt"] = nc.dram_tensor(
            "dbg_vt", [128, PT, NH, 25], FP16, kind="ExternalOutput").ap()
        outs["dbg_rc"] = nc.dram_tensor(
            "dbg_rc", [128, NQT, 4], F32, kind="ExternalOutput").ap()
    with tile.TileContext(nc) as tc:
        emit_kernel(tc, outs, ins)
    nc.compile()
    _PROGRAM = nc
    return nc


def run(inputs, trace=False):
    nc = build_program()
    in_maps = build_inmaps(**inputs)
    try:
        res = bass_utils.run_bass_kernel_spmd(
            nc, in_maps, core_ids=list(range(8)), trace=trace)
    except ModuleNotFoundError:
        # BASS_TRACE path needs antenv.axon_hooks, absent in some containers;
        # rerun untraced rather than failing.
        prev = os.environ.get("BASS_NEVER_TRACE")
        os.environ["BASS_NEVER_TRACE"] = "1"
        try:
            res = bass_utils.run_bass_kernel_spmd(
                nc, in_maps, core_ids=list(range(8)), trace=False)
        finally:
            if prev is None:
                os.environ.pop("BASS_NEVER_TRACE", None)
            else:
                os.environ["BASS_NEVER_TRACE"] = prev
    out_full = np.empty((4, C, HW), dtype=np.float32)
    for core in range(8):
        b, half = core // 2, core % 2
        out_full[b][:, half * QP:(half + 1) * QP] = \
            res.results[core]["out"].astype(np.float32).T
    return out_full.reshape(4, C, 32, 32), res


def kernel(**inputs):
    out, _ = run(inputs, trace=False)
    return out
